# revision 1
# baseline (speedup 1.0000x reference)
"""Trainium2 Bass kernel for nn_MergedConvLiquid.

Model: out = sc + 0.01*(liq - sc) where
  sc  = depthwise causal conv (K=4) over seq,
  liq = per-step gated liquid recurrence with LayerNorm (S sequential steps).

Strategy (8 NeuronCores, SPMD):
  - The recurrence is strictly sequential in time; per-step cost is dominated
    by streaming the recurrent weights W = [W_rec.T | tau_w1_h] (1024x1536)
    through the PE as fp16 stationary tiles (96 tiles of 128x128, FWL).
  - Batch rows are independent: core c runs row c%4 end-to-end (recurrence +
    conv + blend). Host gathers rows 0..3 from cores 0..3.
  - Lazy normalization: carry zero-mean h_hat in fp16; the 1/sqrt(var) scale r
    is folded into the next step's tanh input, so the two per-step inverses
    (1/tau via DVE reciprocal, rsqrt via Newton) run off the critical path,
    overlapped with the PE weight stream.
  - Cross-partition reductions (LN stats, tau dot) use an all-ones 128x128
    stationary matmul, which replicates the sums across all partitions so
    every later scalar op is a [128,1] per-partition op (no broadcasts).
"""

import numpy as np

B, S, H, K = 4, 2048, 1024, 4
DT_, TAU_MIN, TAU_MAX = 0.1, 1.0, 5.0
SCALE = 0.01
LN_EPS = 1e-5
HID = H // 2          # tau hidden width (512)
NJT = H // 128        # 8 j-tiles for H
NHT = HID // 128      # 4 tiles for tau hidden
NKT = H // 128        # 8 k-tiles
NCT = NJT + NHT       # 12 column tiles of Wfull


def _build_kernel(n_steps, unroll=8):
    from concourse import bass, mybir
    from concourse.engine_type import EngineType

    DT = mybir.dt
    AF = mybir.ActivationFunctionType
    OP = mybir.AluOpType
    ds = bass.ds

    def kernel_fn(tc, outs, ins):
        nc = tc.nc
        out_d = outs["out"]
        f32, f16 = DT.float32, DT.float16

        with tc.tile_pool(name="state", bufs=1) as st, \
             tc.tile_pool(name="scr", bufs=2) as scr, \
             tc.tile_pool(name="big", bufs=1) as big, \
             tc.tile_pool(name="scalars", bufs=2) as scp, \
             tc.tile_pool(name="psum_zr", bufs=2, space="PSUM") as p_zr, \
             tc.tile_pool(name="psum_zt", bufs=1, space="PSUM") as p_zt, \
             tc.tile_pool(name="psum_sm", bufs=1, space="PSUM") as p_sm, \
             tc.tile_pool(name="psum_x", bufs=1, space="PSUM") as p_x:

            # ---- persistent SBUF state ----
            wsb = st.tile([128, NKT * NCT * 128], f16)     # W tiles (k,j)
            w1x = st.tile([128, NKT * NHT * 128], f16)     # tau_w1_x tiles
            xt = st.tile([128, NJT * S], f32)              # x row, [p, jt*S + t]
            liqs = st.tile([128, NJT * n_steps], f16)      # h_hat history
            xw1s = st.tile([128, NHT * S], f16)            # x @ tau_w1_x + b1
            rh = st.tile([128, n_steps], f16)              # r history
            hq = st.tile([128, NJT], f16)                  # h_hat carry (fp16)
            r_t = st.tile([128, 1], f32)                   # r carry
            ones = st.tile([128, 128], f32)
            w2 = st.tile([128, NHT], f32)
            b2s = st.tile([128, 1], f32)
            cw = st.tile([128, NJT * K], f32)
            lng = st.tile([128, NJT], f32)
            lnb = st.tile([128, NJT], f32)

            nc.sync.dma_start(wsb[:], ins["wq"][:])
            nc.sync.dma_start(w1x[:], ins["w1x"][:])
            nc.sync.dma_start(xt[:], ins["xt"][:])
            nc.sync.dma_start(ones[:], ins["ones"][:])
            nc.sync.dma_start(w2[:], ins["w2"][:])
            nc.sync.dma_start(b2s[:], ins["b2"][:])
            nc.sync.dma_start(cw[:], ins["cw"][:])
            nc.sync.dma_start(lng[:], ins["lng"][:])
            nc.sync.dma_start(lnb[:], ins["lnb"][:])
            nc.gpsimd.memset(hq[:], 0.0)
            nc.gpsimd.memset(r_t[:], 0.0)

            # ---- precompute xw1 = cast16(x) @ tau_w1_x + b1 ----
            # b1 folded in on host?  No: b1 comes via in 'b1' [128, NHT].
            b1 = st.tile([128, NHT], f32)
            nc.sync.dma_start(b1[:], ins["b1"][:])
            TC = 512
            for tci in range(S // TC):
                xq = big.tile([128, NKT * TC], f16, tag="xq")
                for k in range(NKT):
                    nc.vector.tensor_copy(
                        xq[:, k * TC:(k + 1) * TC],
                        xt[:, k * S + tci * TC: k * S + tci * TC + TC])
                for hti in range(NHT):
                    px = p_x.tile([128, TC], f32, tag="px")
                    for k in range(NKT):
                        nc.tensor.matmul(
                            px[:],
                            w1x[:, (k * NHT + hti) * 128:(k * NHT + hti) * 128 + 128],
                            xq[:, k * TC:(k + 1) * TC],
                            start=(k == 0), stop=(k == NKT - 1))
                    nc.vector.tensor_scalar(
                        xw1s[:, hti * S + tci * TC: hti * S + tci * TC + TC],
                        px[:], b1[:, hti:hti + 1], None, OP.add)

            # ---- the recurrence ----
            def step(t):
                # t may be RuntimeValue (dynamic) or python int
                zt = p_zt.tile([128, NHT], f32, tag="zt")
                zr = p_zr.tile([128, NJT], f32, tag="zr")
                ptau = p_sm.tile([128, 1], f32, tag="ptau")
                pst = p_sm.tile([128, 2], f32, tag="pst")

                # PE: tau columns first
                for ci in range(NJT, NCT):
                    for k in range(NKT):
                        nc.tensor.matmul(
                            zt[:, ci - NJT: ci - NJT + 1],
                            wsb[:, (k * NCT + ci) * 128:(k * NCT + ci) * 128 + 128],
                            hq[:, k:k + 1],
                            start=(k == 0), stop=(k == NKT - 1))
                # PE: first 6 rec columns
                for ci in range(0, 6):
                    for k in range(NKT):
                        nc.tensor.matmul(
                            zr[:, ci:ci + 1],
                            wsb[:, (k * NCT + ci) * 128:(k * NCT + ci) * 128 + 128],
                            hq[:, k:k + 1],
                            start=(k == 0), stop=(k == NKT - 1))

                # tau chain (overlaps PE rec stream)
                ut = scr.tile([128, NHT], f32, tag="ut")
                nc.vector.scalar_tensor_tensor(
                    ut[:], zt[:], r_t[:], xw1s[:, ds(t, NHT, S)],
                    OP.mult, OP.add)
                tu = scr.tile([128, NHT], f32, tag="tu")
                nc.scalar.activation(tu[:], ut[:], AF.Tanh)
                junk4 = scr.tile([128, NHT], f32, tag="junk4")
                taud = scr.tile([128, 1], f32, tag="taud")
                # NOTE: tensor_tensor_reduce faults the exec unit on HW; use
                # a plain mult + tensor_reduce pair instead.
                nc.vector.tensor_tensor(junk4[:], tu[:], w2[:], OP.mult)
                nc.vector.tensor_reduce(taud[:], junk4[:],
                                        mybir.AxisListType.X, OP.add)
                nc.tensor.matmul(ptau[:], ones[:], taud[:], start=True, stop=True)

                # PE: last 2 rec columns (emitted after the tau ones-matmul so
                # the PE keeps streaming while the DVE tau chain runs)
                for ci in range(6, NJT):
                    for k in range(NKT):
                        nc.tensor.matmul(
                            zr[:, ci:ci + 1],
                            wsb[:, (k * NCT + ci) * 128:(k * NCT + ci) * 128 + 128],
                            hq[:, k:k + 1],
                            start=(k == 0), stop=(k == NKT - 1))

                traw = scp.tile([128, 1], f32, tag="traw")
                nc.scalar.activation(traw[:], ptau[:], AF.Sigmoid, bias=b2s[:])
                tau = scp.tile([128, 1], f32, tag="tau")
                nc.vector.tensor_scalar(tau[:], traw[:], TAU_MAX - TAU_MIN,
                                        TAU_MIN, OP.mult, OP.add)
                itau = scp.tile([128, 1], f32, tag="itau")
                nc.vector.reciprocal(itau[:], tau[:])
                lam = scp.tile([128, 1], f32, tag="lam")
                nc.vector.tensor_scalar(lam[:], itau[:], DT_, None, OP.mult)
                lr = scp.tile([128, 1], f32, tag="lr")
                nc.vector.tensor_tensor(lr[:], lam[:], r_t[:], OP.mult)
                a_t = scp.tile([128, 1], f32, tag="a_t")
                nc.vector.tensor_tensor(a_t[:], r_t[:], lr[:], OP.subtract)

                # f chain (critical)
                u = scr.tile([128, NJT], f32, tag="u")
                nc.vector.scalar_tensor_tensor(
                    u[:], zr[:], r_t[:], xt[:, ds(t, NJT, S)], OP.mult, OP.add)
                f = scr.tile([128, NJT], f32, tag="f")
                nc.scalar.activation(f[:], u[:], AF.Tanh)
                t2 = scr.tile([128, NJT], f32, tag="t2")
                nc.vector.tensor_scalar(t2[:], f[:], lam[:], None, OP.mult)
                h_pre = scr.tile([128, NJT], f32, tag="h_pre")
                stats = scr.tile([128, 2], f32, tag="stats")
                nc.vector.scalar_tensor_tensor(
                    h_pre[:], hq[:], a_t[:], t2[:], OP.mult, OP.add,
                    accum_out=stats[:, 0:1])
                sqj = scr.tile([128, NJT], f32, tag="sqj")
                nc.scalar.activation(sqj[:], h_pre[:], AF.Square,
                                     accum_out=stats[:, 1:2])
                nc.tensor.matmul(pst[:], ones[:], stats[:], start=True, stop=True)
                mu = scp.tile([128, 1], f32, tag="mu")
                nc.vector.tensor_scalar(mu[:], pst[:, 0:1], 1.0 / H, None, OP.mult)
                # critical write: new h_hat carry (fp16)
                nc.vector.tensor_scalar(hq[:], h_pre[:], mu[:], None, OP.subtract)
                # history write (off critical path)
                nc.vector.tensor_scalar(liqs[:, ds(t, NJT, n_steps)], h_pre[:],
                                        mu[:], None, OP.subtract)

                # var + Newton rsqrt (off critical path; result used next step)
                m2 = scp.tile([128, 1], f32, tag="m2")
                nc.vector.tensor_scalar(m2[:], pst[:, 1:2], 1.0 / H, None, OP.mult)
                musq = scp.tile([128, 1], f32, tag="musq")
                nc.vector.tensor_tensor(musq[:], mu[:], mu[:], OP.mult)
                v = scp.tile([128, 1], f32, tag="v")
                nc.vector.tensor_tensor(v[:], m2[:], musq[:], OP.subtract)
                nc.vector.tensor_scalar(v[:], v[:], LN_EPS, None, OP.add)
                # rsqrt via Quake seed (bit trick) + 3 Newton iterations;
                # valid over the full var range incl. the tiny first-step var.
                y = scp.tile([128, 1], f32, tag="y")
                i32 = DT.int32
                nc.vector.tensor_scalar(y[:].bitcast(i32), v[:].bitcast(i32),
                                        1, None, OP.logical_shift_right)
                nc.vector.tensor_scalar(y[:].bitcast(i32), y[:].bitcast(i32),
                                        -1, 0x5F3759DF, OP.mult, OP.add)
                tn = scp.tile([128, 1], f32, tag="tn")
                for _ in range(3):
                    nc.vector.tensor_tensor(tn[:], y[:], y[:], OP.mult)
                    nc.vector.tensor_tensor(tn[:], tn[:], v[:], OP.mult)
                    nc.vector.tensor_scalar(tn[:], tn[:], -0.5, 1.5, OP.mult, OP.add)
                    nc.vector.tensor_tensor(y[:], y[:], tn[:], OP.mult)
                nc.vector.tensor_copy(r_t[:], y[:])
                nc.vector.tensor_copy(rh[:, ds(t, 1)], r_t[:])

            if unroll <= 1 or n_steps <= unroll:
                for t in range(n_steps):
                    step(t)
            else:
                assert n_steps % unroll == 0
                with tc.For_i(0, n_steps, unroll,
                              hint_engines=(EngineType.PE, EngineType.DVE,
                                            EngineType.Activation)) as iv:
                    for u in range(unroll):
                        step(iv + u)

            # ---- conv + blend + output ----
            for jt in range(NJT):
                sc = big.tile([128, S], f32, tag="sc")
                # k = K-1 (no shift) initializes
                nc.vector.tensor_scalar(
                    sc[:], xt[:, jt * S:(jt + 1) * S],
                    cw[:, jt * K + (K - 1): jt * K + K], None, OP.mult)
                for k in range(K - 1):
                    sh = K - 1 - k  # left shift amount
                    nc.vector.scalar_tensor_tensor(
                        sc[:, sh:S], xt[:, jt * S: (jt + 1) * S - sh],
                        cw[:, jt * K + k: jt * K + k + 1], sc[:, sh:S],
                        OP.mult, OP.add)
                # liq_n = (h_hat * r) * g + b
                lq = big.tile([128, S], f32, tag="lq")
                if n_steps == S:
                    nc.vector.tensor_tensor(
                        lq[:], liqs[:, jt * n_steps:(jt + 1) * n_steps], rh[:],
                        OP.mult)
                else:
                    nc.vector.memset(lq[:], 0.0)
                    nc.vector.tensor_tensor(
                        lq[:, 0:n_steps], liqs[:, jt * n_steps:(jt + 1) * n_steps],
                        rh[:], OP.mult)
                nc.vector.tensor_scalar(
                    lq[:], lq[:], lng[:, jt:jt + 1], lnb[:, jt:jt + 1],
                    OP.mult, OP.add)
                nc.vector.tensor_scalar(sc[:], sc[:], 1.0 - SCALE, None, OP.mult)
                nc.vector.scalar_tensor_tensor(
                    sc[:], lq[:], SCALE, sc[:], OP.mult, OP.add)
                nc.sync.dma_start(out_d[jt], sc[:])

    return kernel_fn


def _prep_in_maps(hidden_states, conv_w, W_rec, tau_w1, tau_b1, tau_w2, tau_b2,
                  ln_g, ln_b):
    """Host-side staging: per-core input dict (core c gets batch row c%4)."""
    x = np.asarray(hidden_states, dtype=np.float32)
    Wfull = np.concatenate([np.asarray(W_rec).T, np.asarray(tau_w1)[H:]], axis=1)
    # [kk, kt, ct, jj]
    wq = Wfull.reshape(NKT, 128, NCT, 128).transpose(1, 0, 2, 3)
    wq = np.ascontiguousarray(wq, dtype=np.float16).reshape(128, NKT * NCT * 128)
    w1xh = np.asarray(tau_w1)[:H]  # [H, HID]
    w1x = w1xh.reshape(NKT, 128, NHT, 128).transpose(1, 0, 2, 3)
    w1x = np.ascontiguousarray(w1x, dtype=np.float16).reshape(128, NKT * NHT * 128)
    w2 = np.ascontiguousarray(
        np.asarray(tau_w2)[:, 0].reshape(NHT, 128).T, dtype=np.float32)
    b1 = np.ascontiguousarray(
        np.asarray(tau_b1).reshape(NHT, 128).T, dtype=np.float32)
    b2 = np.full((128, 1), float(np.asarray(tau_b2)[0]), dtype=np.float32)
    cw = np.ascontiguousarray(
        np.asarray(conv_w).reshape(NJT, 128, K).transpose(1, 0, 2),
        dtype=np.float32).reshape(128, NJT * K)
    lng = np.ascontiguousarray(
        np.asarray(ln_g).reshape(NJT, 128).T, dtype=np.float32)
    lnb = np.ascontiguousarray(
        np.asarray(ln_b).reshape(NJT, 128).T, dtype=np.float32)
    ones = np.ones((128, 128), dtype=np.float32)

    shared = dict(wq=wq, w1x=w1x, w2=w2, b1=b1, b2=b2, cw=cw, lng=lng, lnb=lnb,
                  ones=ones)
    in_maps = []
    for c in range(8):
        b = c % B
        xtb = np.ascontiguousarray(
            x[b].T.reshape(NJT, 128, S).transpose(1, 0, 2),
            dtype=np.float32).reshape(128, NJT * S)
        m = dict(shared)
        m["xt"] = xtb
        in_maps.append(m)
    return in_maps


_IN_SPECS = None


def _in_specs():
    from concourse import mybir
    DT = mybir.dt
    return {
        "wq": ((128, NKT * NCT * 128), DT.float16),
        "w1x": ((128, NKT * NHT * 128), DT.float16),
        "xt": ((128, NJT * S), DT.float32),
        "w2": ((128, NHT), DT.float32),
        "b1": ((128, NHT), DT.float32),
        "b2": ((128, 1), DT.float32),
        "cw": ((128, NJT * K), DT.float32),
        "lng": ((128, NJT), DT.float32),
        "lnb": ((128, NJT), DT.float32),
        "ones": ((128, 128), DT.float32),
    }


def _run_spmd(kernel_fn, in_specs, out_specs, in_maps, num_cores=8, trace=False):
    from concourse import bacc, tile
    from concourse.bass_interp import MultiCoreSim

    nc = bacc.Bacc(
        "TRN2",
        target_bir_lowering=False,
        debug=False,
        enable_asserts=True,
        num_devices=num_cores,
    )
    in_tiles = {
        name: nc.dram_tensor(name, list(shape), dt, kind="ExternalInput").ap()
        for name, (shape, dt) in in_specs.items()
    }
    out_tiles = {
        name: nc.dram_tensor(name, list(shape), dt, kind="ExternalOutput").ap()
        for name, (shape, dt) in out_specs.items()
    }
    with tile.TileContext(nc, trace_sim=True) as tc:
        kernel_fn(tc, out_tiles, in_tiles)
    nc.compile()

    sim = MultiCoreSim(nc, num_cores=num_cores, trace=True)
    for i, core in sim.cores.items():
        for name, arr in in_maps[i].items():
            core.tensor(name)[:] = arr
    return sim.run_on_hw_raw(trace=trace)


def run_on_device(inputs, n_steps=S, unroll=8, trace=False):
    from concourse import mybir
    DT = mybir.dt
    in_maps = _prep_in_maps(**inputs)
    kernel_fn = _build_kernel(n_steps, unroll=unroll)
    out_specs = {"out": ((NJT, 128, S), DT.float32)}
    res = _run_spmd(kernel_fn, _in_specs(), out_specs, in_maps, trace=trace)
    outs = np.empty((B, S, H), dtype=np.float32)
    for b in range(B):
        o = res.results[b]["out"]  # [NJT, 128, S]
        outs[b] = o.reshape(H, S).T
    return outs, res


def kernel(**inputs):
    out, _ = run_on_device(inputs)
    return out



# revision 17
# speedup vs baseline: 1.6327x; 1.6327x over previous
"""Trainium2 Bass kernel for nn_MergedConvLiquid (v2).

Model: out = sc + 0.01*(liq - sc) where
  sc  = depthwise causal conv (K=4) over seq,
  liq = per-step gated liquid recurrence with LayerNorm (S sequential steps).

v2 strategy (vs baseline): keep one batch row per core (cores 0-3, with 4-7
duplicating), but restructure the recurrence so the PE weight stream is the
only serial resource and every other engine (DVE / Act / GpSimd) runs off
the critical path:

  - Unnormalized carry c (fp16): LayerNorm's mean subtraction is dropped
    from the state (validated: bounded drift, exact at output via the
    rho/mu histories); the 1/sqrt(var) scale rho is applied lazily.
  - rho(t+1) = one warm-started Newton step from rho(t) toward
    rsqrt(var(t-1) + eps): one-step-lagged LN scale, so the scale for the
    next matvec is ready mid-stream instead of serializing on this step's
    statistics. Error ~ per-step var drift (<6%) -> ~7e-4 on the output.
  - tau MLP runs one step ahead on h(t-1) ("stale tau", error ~3e-5):
    lambda/a for step t+1 are computed during stream t, so the chain
    f->c' never waits on the tau path.
  - Cross-partition sums (tau dot, LN stats) via fp16 ones-matmuls placed
    late in the PE stream so the PE never stalls on them.
  - sigmoid(x) computed as 0.5*(1+tanh(x/2)) so the Act engine only needs
    {tanh, square} (one activation table - no ACT_TABLE_LOAD thrash).
  - conv + blend + output assembly hidden inside the steady loop (column
    t-8 finalized during step t) on spare DVE/GpSimd cycles.
"""

import numpy as np

B, S, H, K = 4, 2048, 1024, 4
DT_, TAU_MIN, TAU_MAX = 0.1, 1.0, 5.0
SCALE = 0.01
LN_EPS = 1e-5
HID = H // 2          # tau hidden width (512)
NJT = H // 128        # 8 j-tiles for H
NHT = HID // 128      # 4 tiles for tau hidden
NKT = H // 128        # 8 k-tiles
NCT = NJT + NHT       # 12 column tiles of Wfull
PRO = 8               # prologue steps (exact rho)


def _build_kernel(n_steps, unroll=8, w_dt_name="float16"):
    from concourse import bass, mybir
    from concourse.engine_type import EngineType

    DT = mybir.dt
    AF = mybir.ActivationFunctionType
    OP = mybir.AluOpType
    AX = mybir.AxisListType
    ds = bass.ds

    assert (n_steps - PRO) % unroll == 0

    def kernel_fn(tc, outs, ins):
        nc = tc.nc
        f32, f16 = DT.float32, DT.float16
        wdt = getattr(DT, w_dt_name)
        SP1 = S + 1

        with tc.tile_pool(name="state", bufs=1) as st, \
             tc.tile_pool(name="scr", bufs=2) as scr, \
             tc.tile_pool(name="psum_zr", bufs=2, space="PSUM") as p_zr, \
             tc.tile_pool(name="psum_zt", bufs=2, space="PSUM") as p_zt, \
             tc.tile_pool(name="psum_sm", bufs=2, space="PSUM") as p_sm, \
             tc.tile_pool(name="psum_x", bufs=2, space="PSUM") as p_x:

            # ---- persistent SBUF state ----
            wsb = st.tile([128, NKT * NCT * 128], wdt)      # W tiles (k,ci)
            w1x = st.tile([128, NKT * NHT * 128], f16)      # tau_w1_x tiles
            xt = st.tile([128, NJT * S], f16)               # x row [p, jt*S+t]
            sc16 = st.tile([128, NJT * S], f16)             # conv result
            outb = st.tile([128, NJT * S], f16)             # final output
            liqs = st.tile([128, NJT * SP1], f16)           # c history (slot t+1)
            xw1s = st.tile([128, NHT * SP1], f16)           # x@tau_w1_x + b1
            rh = st.tile([128, S], f32)                     # rho(t+1) history
            rmuh = st.tile([128, S], f32)                   # -rho(t+1)*mu(t) hist
            hq = [st.tile([128, NJT], f16, name=f"hq{i}") for i in range(2)]
            rho = [st.tile([128, 1], f32, name=f"rho{i}") for i in range(2)]
            lam = [st.tile([128, 1], f32, name=f"lam{i}") for i in range(2)]
            at = [st.tile([128, 1], f32, name=f"at{i}") for i in range(2)]
            stt_ = [st.tile([128, 2], f32, name=f"stt{i}") for i in range(2)]
            st16 = [st.tile([128, 2], f16, name=f"st16_{i}") for i in range(2)]
            ones = st.tile([128, 128], f16)
            w2 = st.tile([128, NHT], f32)
            b2h = st.tile([128, 1], f32)                    # 0.5*b2
            b1 = st.tile([128, NHT], f32)
            cw = st.tile([128, NJT * K], f32)
            lng = st.tile([128, NJT], f32)
            lnb = st.tile([128, NJT], f32)

            nc.sync.dma_start(wsb[:], ins["wq"][:])
            nc.sync.dma_start(w1x[:], ins["w1x"][:])
            nc.sync.dma_start(xt[:], ins["xt"][:])
            nc.sync.dma_start(ones[:], ins["ones"][:])
            nc.sync.dma_start(w2[:], ins["w2"][:])
            nc.sync.dma_start(b2h[:], ins["b2h"][:])
            nc.sync.dma_start(b1[:], ins["b1"][:])
            nc.sync.dma_start(cw[:], ins["cw"][:])
            nc.sync.dma_start(lng[:], ins["lng"][:])
            nc.sync.dma_start(lnb[:], ins["lnb"][:])
            nc.gpsimd.memset(hq[0][:], 0.0)
            nc.gpsimd.memset(rho[0][:], 1.0)
            nc.vector.memset(liqs[:, ds(0, NJT, SP1)], 0.0)
            if n_steps < S:
                nc.gpsimd.memset(outb[:], 0.0)   # test mode: unwritten tail
                nc.gpsimd.memset(rh[:], 0.0)
                nc.gpsimd.memset(rmuh[:], 0.0)
                nc.gpsimd.memset(liqs[:], 0.0)

            # ---- xw1s = cast16(x) @ tau_w1_x + b1 (stride S+1 layout) ----
            TC = 512
            for tci in range(S // TC):
                for hti in range(NHT):
                    px = p_x.tile([128, TC], f32, tag="px")
                    for k in range(NKT):
                        nc.tensor.matmul(
                            px[:],
                            w1x[:, (k * NHT + hti) * 128:(k * NHT + hti) * 128 + 128],
                            xt[:, k * S + tci * TC: k * S + tci * TC + TC],
                            start=(k == 0), stop=(k == NKT - 1))
                    nc.vector.tensor_scalar(
                        xw1s[:, hti * SP1 + tci * TC: hti * SP1 + tci * TC + TC],
                        px[:], b1[:, hti:hti + 1], None, OP.add)
            nc.vector.memset(xw1s[:, ds(S, NHT, SP1)], 0.0)

            # ---- conv precompute: sc16[jt] (DVE for jt 0..4, gpsimd 5..7) ----
            for jt in range(NJT):
                eng = nc.vector
                xs = xt[:, jt * S:(jt + 1) * S]
                scs = sc16[:, jt * S:(jt + 1) * S]
                eng.tensor_scalar(
                    scs, xs, cw[:, jt * K + (K - 1): jt * K + K], None, OP.mult)
                for k in range(K - 1):
                    sh = K - 1 - k
                    eng.scalar_tensor_tensor(
                        scs[:, sh:S], xt[:, jt * S: (jt + 1) * S - sh],
                        cw[:, jt * K + k: jt * K + k + 1], scs[:, sh:S],
                        OP.mult, OP.add)

            def quake_rsqrt(out_ap, v_ap, iters=3):
                """exact-ish rsqrt on DVE (prologue only): quake seed + NR."""
                i32 = DT.int32
                y = scr.tile([128, 1], f32, tag="qk_y")
                tn = scr.tile([128, 1], f32, tag="qk_t")
                nc.vector.tensor_scalar(y[:].bitcast(i32), v_ap.bitcast(i32),
                                        1, None, OP.logical_shift_right)
                nc.vector.tensor_scalar(y[:].bitcast(i32), y[:].bitcast(i32),
                                        -1, 0x5F3759DF, OP.mult, OP.add)
                for _ in range(iters):
                    nc.vector.tensor_tensor(tn[:], y[:], y[:], OP.mult)
                    nc.vector.tensor_tensor(tn[:], tn[:], v_ap, OP.mult)
                    nc.vector.tensor_scalar(tn[:], tn[:], -0.5, 1.5, OP.mult, OP.add)
                    nc.vector.tensor_tensor(y[:], y[:], tn[:], OP.mult)
                nc.vector.tensor_copy(out_ap, y[:])

            def mm(out_ap, ci, k, hq_c, start, stop):
                nc.tensor.matmul(
                    out_ap,
                    wsb[:, (k * NCT + ci) * 128:(k * NCT + ci) * 128 + 128],
                    hq_c[:, k:k + 1],
                    start=start, stop=stop, skip_group_check=True)

            last_u1 = [None, None]

            def step(t, par, exact_rho=False, do_final=True, handoff=False):
                """Emit one step. t may be python int (prologue) or RuntimeValue."""
                hq_c, hq_n = hq[par], hq[1 - par]
                rho_c, rho_n = rho[par], rho[1 - par]
                lam_c, lam_n = lam[par], lam[1 - par]
                a_c, a_n = at[par], at[1 - par]
                st_c, st_p = stt_[par], stt_[1 - par]
                st16_c, st16_p = st16[par], st16[1 - par]

                zr = p_zr.tile([128, NJT], f32, tag="zr")
                zt = p_zt.tile([128, NHT], f32, tag="zt")
                sm = p_sm.tile([128, 4], f32, tag="sm")
                ptau = sm[:, 0:1]
                pst = sm[:, 1:3]

                # === prologue-only: exact rho fix for THIS step (needs stats(t-1))
                if exact_rho:
                    # stats-MM(t-1)
                    nc.tensor.matmul(pst, ones[:], st16_p[:],
                                     start=True, stop=True)
                    ps = scr.tile([128, 2], f32, tag="psx")
                    mneg = scr.tile([128, 1], f32, tag="mnegx")
                    vp = scr.tile([128, 1], f32, tag="vpx")
                    nc.vector.tensor_copy(ps[:], pst)
                    nc.vector.tensor_scalar(mneg[:], ps[:, 0:1], -1.0 / H,
                                            None, OP.mult)
                    nc.vector.scalar_tensor_tensor(vp[:], ps[:, 0:1], mneg[:],
                                                   ps[:, 1:2], OP.mult, OP.add)
                    nc.vector.tensor_scalar(vp[:], vp[:], 1.0 / H, LN_EPS,
                                            OP.mult, OP.add)
                    quake_rsqrt(rho_c[:], vp[:])  # rho(t) exact
                    nc.vector.tensor_copy(rh[:, ds(t - 1, 1)], rho_c[:])
                    nc.vector.tensor_tensor(rmuh[:, ds(t - 1, 1)], rho_c[:],
                                            mneg[:], OP.mult)
                    # a(t) = rho(t) - lam(t)*rho(t)
                    lrx = scr.tile([128, 1], f32, tag="lrx")
                    nc.vector.tensor_tensor(lrx[:], lam_c[:], rho_c[:], OP.mult)
                    nc.vector.tensor_tensor(a_c[:], rho_c[:], lrx[:], OP.subtract)
                    if handoff:
                        # steady-style rho(t+1) = NR1(rho(t) -> var(t-1)+eps)
                        r2 = scr.tile([128, 1], f32, tag="r2x")
                        nc.vector.tensor_tensor(r2[:], rho_c[:], rho_c[:], OP.mult)
                        nc.vector.tensor_tensor(r2[:], r2[:], vp[:], OP.mult)
                        nc.vector.tensor_scalar(r2[:], r2[:], -0.5, 1.5,
                                                OP.mult, OP.add)
                        nc.vector.tensor_tensor(rho_n[:], rho_c[:], r2[:], OP.mult)
                        nc.vector.tensor_copy(rh[:, ds(t, 1)], rho_n[:])

                # === PE: rec-a (ci 0..3)  [BISECT: no k-split]
                for ci in range(4):
                    for k in range(8):
                        mm(zr[:, ci:ci + 1], ci, k, hq_c, k == 0, k == 7)

                # === PE: tau block (z_tau for step t+1)
                for ci in range(NJT, NCT):
                    for k in range(NKT):
                        mm(zt[:, ci - NJT:ci - NJT + 1], ci, k, hq_c,
                           k == 0, k == 7)

                # === stats-MM(t-1) + rho chain (steady only; prologue did exact)
                skip_stats = isinstance(t, int) and t == 0
                if not exact_rho and not skip_stats:
                    nc.tensor.matmul(pst, ones[:], st16_p[:],
                                     start=True, stop=True)
                    ps = scr.tile([128, 2], f32, tag="ps")
                    mneg = scr.tile([128, 1], f32, tag="mneg")
                    vp = scr.tile([128, 1], f32, tag="vp")
                    nc.vector.tensor_copy(ps[:], pst)
                    nc.gpsimd.tensor_scalar(mneg[:], ps[:, 0:1], -1.0 / H,
                                            None, OP.mult)
                    nc.vector.scalar_tensor_tensor(vp[:], ps[:, 0:1], mneg[:],
                                                   ps[:, 1:2], OP.mult, OP.add)
                    nc.gpsimd.tensor_scalar(vp[:], vp[:], 1.0 / H, LN_EPS,
                                            OP.mult, OP.add)
                    # NR1: rho_n = rho_c*(1.5 - 0.5*vp*rho_c^2)
                    r2 = scr.tile([128, 1], f32, tag="r2")
                    nc.gpsimd.tensor_tensor(r2[:], rho_c[:], rho_c[:], OP.mult)
                    nc.gpsimd.tensor_tensor(r2[:], r2[:], vp[:], OP.mult)
                    nc.gpsimd.tensor_scalar(r2[:], r2[:], -0.5, 1.5,
                                            OP.mult, OP.add)
                    nc.gpsimd.tensor_tensor(rho_n[:], rho_c[:], r2[:], OP.mult)
                    nc.gpsimd.tensor_copy(rh[:, ds(t, 1)], rho_n[:])
                    nc.gpsimd.tensor_tensor(rmuh[:, ds(t - 1, 1)], rho_c[:],
                                            mneg[:], OP.mult)

                # === tau chain for step t+1 (stale h)
                u_tau = scr.tile([128, NHT], f32, tag="ut")
                tu = scr.tile([128, NHT], f16, tag="tu")
                junk = scr.tile([128, NHT], f32, tag="junk")
                taud = scr.tile([128, 1], f16, tag="taud")
                nc.vector.scalar_tensor_tensor(
                    u_tau[:], zt[:], rho_c[:], xw1s[:, ds(t + 1, NHT, SP1)],
                    OP.mult, OP.add)
                nc.scalar.activation(tu[:], u_tau[:], AF.Tanh)
                nc.vector.tensor_tensor(junk[:], tu[:], w2[:], OP.mult)
                with nc.allow_low_precision(reason="4-elem tau dot, fp16 ample"):
                    nc.vector.tensor_reduce(taud[:], junk[:], AX.X, OP.add)

                # === PE: rec-b (ci 4..7)
                for ci in range(4, 8):
                    for k in range(NKT):
                        mm(zr[:, ci:ci + 1], ci, k, hq_c, k == 0, k == 7)

                # === taud ones-MM
                nc.tensor.matmul(ptau, ones[:], taud[:], start=True, stop=True)

                # === DVE chain: u1 -> f -> q -> c'
                u1 = scr.tile([128, NJT], f32, tag="u1")
                last_u1[0] = u1
                ff = scr.tile([128, NJT], f32, tag="ff")
                qq = scr.tile([128, NJT], f32, tag="qq")
                nc.vector.scalar_tensor_tensor(
                    u1[:], zr[:], rho_c[:], xt[:, ds(t, NJT, S)],
                    OP.mult, OP.add)
                nc.scalar.activation(ff[:], u1[:], AF.Tanh)
                nc.vector.tensor_scalar(qq[:], ff[:], lam_c[:], None, OP.mult)
                nc.vector.scalar_tensor_tensor(
                    hq_n[:], hq_c[:], a_c[:], qq[:], OP.mult, OP.add,
                    accum_out=st_c[:, 0:1])
                nc.vector.tensor_copy(liqs[:, ds(t + 1, NJT, SP1)], hq_n[:])
                sq = scr.tile([128, NJT], f16, tag="sq")
                nc.scalar.activation(sq[:], hq_n[:], AF.Square,
                                     accum_out=st_c[:, 1:2])
                nc.vector.tensor_copy(st16_c[:], st_c[:, 0:2])

                # === sigmoid chain -> lam(t+1), a(t+1)
                th = scr.tile([128, 1], f32, tag="th")
                tauv = scr.tile([128, 1], f32, tag="tauv")
                itau = scr.tile([128, 1], f32, tag="itau")
                lr = scr.tile([128, 1], f32, tag="lr")
                nc.scalar.activation(th[:], ptau, AF.Tanh,
                                     bias=b2h[:], scale=0.5)
                nc.gpsimd.tensor_scalar(tauv[:], th[:], 2.0, 3.0, OP.mult, OP.add)
                nc.vector.reciprocal(itau[:], tauv[:])
                nc.gpsimd.tensor_scalar(lam_n[:], itau[:], DT_, None, OP.mult)
                if not handoff and not exact_rho and not skip_stats:
                    nc.gpsimd.tensor_tensor(lr[:], lam_n[:], rho_n[:], OP.mult)
                    nc.gpsimd.tensor_tensor(a_n[:], rho_n[:], lr[:], OP.subtract)
                elif handoff:
                    nc.vector.tensor_tensor(lr[:], lam_n[:], rho_n[:], OP.mult)
                    nc.vector.tensor_tensor(a_n[:], rho_n[:], lr[:], OP.subtract)
                # (plain prologue: a(t+1) computed by next prologue step's
                #  exact-rho fix; lam_n is stored for it.)

                # === hidden final for column t' = t - PRO
                if do_final:
                    tp = t - PRO
                    lq8 = scr.tile([128, NJT], f32, tag="lq8")
                    m1 = scr.tile([128, NJT], f32, tag="m1")
                    o1 = scr.tile([128, NJT], f32, tag="o1")
                    nc.vector.tensor_scalar(
                        lq8[:], liqs[:, ds(tp + 1, NJT, SP1)],
                        rh[:, ds(tp, 1)], rmuh[:, ds(tp, 1)], OP.mult, OP.add)
                    nc.gpsimd.tensor_tensor(m1[:], lq8[:], lng[:], OP.mult)
                    nc.gpsimd.tensor_tensor(m1[:], m1[:], lnb[:], OP.add)
                    nc.gpsimd.tensor_scalar(o1[:], m1[:], SCALE, None, OP.mult)
                    nc.vector.scalar_tensor_tensor(
                        outb[:, ds(tp, NJT, S)], sc16[:, ds(tp, NJT, S)],
                        1.0 - SCALE, o1[:], OP.mult, OP.add)

            # ---- bootstrap: lam(0), a(0) from xw1s col 0 (z_tau(-1) = 0) ----
            tu0 = scr.tile([128, NHT], f16, tag="tu0")
            junk0 = scr.tile([128, NHT], f32, tag="junk0")
            taud0 = scr.tile([128, 1], f16, tag="taud0")
            sm0 = p_sm.tile([128, 4], f32, tag="sm")
            pt0 = sm0[:, 0:1]
            th0 = scr.tile([128, 1], f32, tag="th0")
            tv0 = scr.tile([128, 1], f32, tag="tv0")
            it0 = scr.tile([128, 1], f32, tag="it0")
            nc.scalar.activation(tu0[:], xw1s[:, ds(0, NHT, SP1)], AF.Tanh)
            nc.vector.tensor_tensor(junk0[:], tu0[:], w2[:], OP.mult)
            with nc.allow_low_precision(reason="4-elem tau dot, fp16 ample"):
                nc.vector.tensor_reduce(taud0[:], junk0[:], AX.X, OP.add)
            nc.tensor.matmul(pt0, ones[:], taud0[:], start=True, stop=True)
            nc.scalar.activation(th0[:], pt0, AF.Tanh, bias=b2h[:], scale=0.5)
            nc.vector.tensor_scalar(tv0[:], th0[:], 2.0, 3.0, OP.mult, OP.add)
            nc.vector.reciprocal(it0[:], tv0[:])
            nc.vector.tensor_scalar(lam[0][:], it0[:], DT_, None, OP.mult)
            nc.vector.tensor_scalar(at[0][:], lam[0][:], -1.0, 1.0,
                                    OP.mult, OP.add)

            # ---- prologue steps 0..PRO-1 ----
            for t in range(PRO):
                step(t, t & 1, exact_rho=(t >= 1), do_final=False,
                     handoff=(t == PRO - 1))

            # ---- steady loop ----
            with tc.For_i(PRO, n_steps, unroll,
                          hint_engines=(EngineType.PE, EngineType.DVE,
                                        EngineType.Activation,
                                        EngineType.Pool)) as iv:
                for u in range(unroll):
                    step(iv + u, (PRO + u) & 1, do_final=True)

            # ---- tail ----
            # stats(n-1) -> rmuh[n-1] = -rho(n)*mu(n-1)
            par_last = (n_steps - 1) & 1          # parity used by last step
            smT = p_sm.tile([128, 4], f32, tag="sm")
            pstT = smT[:, 1:3]
            nc.tensor.matmul(pstT, ones[:], st16[par_last][:],
                             start=True, stop=True)
            mnegT = scr.tile([128, 1], f32, tag="mnegT")
            nc.vector.tensor_scalar(mnegT[:], smT[:, 1:2], -1.0 / H,
                                    None, OP.mult)
            nc.vector.tensor_tensor(rmuh[:, ds(n_steps - 1, 1)],
                                    rho[1 - par_last][:], mnegT[:], OP.mult)
            # final columns n_steps-PRO .. n_steps-1
            for tp in range(n_steps - PRO, n_steps):
                lq8 = scr.tile([128, NJT], f32, tag="lq8T")
                m1 = scr.tile([128, NJT], f32, tag="m1T")
                o1 = scr.tile([128, NJT], f32, tag="o1T")
                nc.vector.tensor_scalar(
                    lq8[:], liqs[:, ds(tp + 1, NJT, SP1)],
                    rh[:, ds(tp, 1)], rmuh[:, ds(tp, 1)], OP.mult, OP.add)
                nc.vector.tensor_tensor(m1[:], lq8[:], lng[:], OP.mult)
                nc.vector.tensor_tensor(m1[:], m1[:], lnb[:], OP.add)
                nc.vector.tensor_scalar(o1[:], m1[:], SCALE, None, OP.mult)
                nc.vector.scalar_tensor_tensor(
                    outb[:, ds(tp, NJT, S)], sc16[:, ds(tp, NJT, S)],
                    1.0 - SCALE, o1[:], OP.mult, OP.add)

            nc.sync.dma_start(outs["outb"], outb[:])
            if "dbg_zr" in outs:
                zdump = st.tile([128, NJT], f32, name="zdump")
                nc.vector.tensor_copy(zdump[:], last_u1[0][:])
                nc.sync.dma_start(outs["dbg_zr"], zdump[:])
            if "dbg_rh" in outs:
                nc.sync.dma_start(outs["dbg_rh"], rh[:])
                nc.sync.dma_start(outs["dbg_rmuh"], rmuh[:])
                nc.sync.dma_start(outs["dbg_liqs"], liqs[:])

    return kernel_fn


def _prep_in_maps(hidden_states, conv_w, W_rec, tau_w1, tau_b1, tau_w2, tau_b2,
                  ln_g, ln_b, w_dt_name="float16"):
    """Host-side staging: per-core input dict (core c gets batch row c%4)."""
    if w_dt_name == "float16":
        np_wdt = np.float16
    else:
        import ml_dtypes
        np_wdt = ml_dtypes.float8_e4m3fn
    x = np.asarray(hidden_states, dtype=np.float32)
    Wfull = np.concatenate([np.asarray(W_rec).T, np.asarray(tau_w1)[H:]], axis=1)
    # [kk, kt, ct, jj]
    wq = Wfull.reshape(NKT, 128, NCT, 128).transpose(1, 0, 2, 3)
    wq = np.ascontiguousarray(wq).astype(np_wdt).reshape(128, NKT * NCT * 128)
    w1xh = np.asarray(tau_w1)[:H]  # [H, HID]
    w1x = w1xh.reshape(NKT, 128, NHT, 128).transpose(1, 0, 2, 3)
    w1x = np.ascontiguousarray(w1x, dtype=np.float16).reshape(128, NKT * NHT * 128)
    w2 = np.ascontiguousarray(
        np.asarray(tau_w2)[:, 0].reshape(NHT, 128).T, dtype=np.float32)
    b1 = np.ascontiguousarray(
        np.asarray(tau_b1).reshape(NHT, 128).T, dtype=np.float32)
    b2h = np.full((128, 1), 0.5 * float(np.asarray(tau_b2)[0]), dtype=np.float32)
    cw = np.ascontiguousarray(
        np.asarray(conv_w).reshape(NJT, 128, K).transpose(1, 0, 2),
        dtype=np.float32).reshape(128, NJT * K)
    lng = np.ascontiguousarray(
        np.asarray(ln_g).reshape(NJT, 128).T, dtype=np.float32)
    lnb = np.ascontiguousarray(
        np.asarray(ln_b).reshape(NJT, 128).T, dtype=np.float32)
    ones = np.ones((128, 128), dtype=np.float16)

    shared = dict(wq=wq, w1x=w1x, w2=w2, b1=b1, b2h=b2h, cw=cw, lng=lng,
                  lnb=lnb, ones=ones)
    in_maps = []
    for c in range(8):
        b = c % B
        xtb = np.ascontiguousarray(
            x[b].T.reshape(NJT, 128, S).transpose(1, 0, 2),
            dtype=np.float16).reshape(128, NJT * S)
        m = dict(shared)
        m["xt"] = xtb
        in_maps.append(m)
    return in_maps


def _in_specs(w_dt_name="float16"):
    from concourse import mybir
    DT = mybir.dt
    wdt = getattr(DT, w_dt_name)
    return {
        "wq": ((128, NKT * NCT * 128), wdt),
        "w1x": ((128, NKT * NHT * 128), DT.float16),
        "xt": ((128, NJT * S), DT.float16),
        "w2": ((128, NHT), DT.float32),
        "b1": ((128, NHT), DT.float32),
        "b2h": ((128, 1), DT.float32),
        "cw": ((128, NJT * K), DT.float32),
        "lng": ((128, NJT), DT.float32),
        "lnb": ((128, NJT), DT.float32),
        "ones": ((128, 128), DT.float16),
    }


def _run_spmd(kernel_fn, in_specs, out_specs, in_maps, num_cores=8, trace=False,
              sim_only=False):
    from concourse import bacc, tile
    from concourse.bass_interp import MultiCoreSim

    nc = bacc.Bacc(
        "TRN2",
        target_bir_lowering=False,
        debug=False,
        enable_asserts=True,
        num_devices=num_cores,
    )
    in_tiles = {
        name: nc.dram_tensor(name, list(shape), dt, kind="ExternalInput").ap()
        for name, (shape, dt) in in_specs.items()
    }
    out_tiles = {
        name: nc.dram_tensor(name, list(shape), dt, kind="ExternalOutput").ap()
        for name, (shape, dt) in out_specs.items()
    }
    with tile.TileContext(nc, trace_sim=True) as tc:
        kernel_fn(tc, out_tiles, in_tiles)
    nc.compile()

    sim = MultiCoreSim(nc, num_cores=num_cores, trace=True)
    for i, core in sim.cores.items():
        for name, arr in in_maps[i].items():
            core.tensor(name)[:] = arr
    if sim_only:
        sim.simulate()
        return sim
    return sim.run_on_hw_raw(trace=trace)


def run_on_device(inputs, n_steps=S, unroll=8, trace=False, w_dt_name="float16",
                  sim_only=False, num_cores=8):
    from concourse import mybir
    DT = mybir.dt
    in_maps = _prep_in_maps(**inputs, w_dt_name=w_dt_name)[:num_cores]
    kernel_fn = _build_kernel(n_steps, unroll=unroll, w_dt_name=w_dt_name)
    out_specs = {"outb": ((128, NJT * S), DT.float16)}
    if sim_only:
        out_specs["dbg_zr"] = ((128, NJT), DT.float32)
        out_specs["dbg_rh"] = ((128, S), DT.float32)
        out_specs["dbg_rmuh"] = ((128, S), DT.float32)
        out_specs["dbg_liqs"] = ((128, NJT * (S + 1)), DT.float16)
    res = _run_spmd(kernel_fn, _in_specs(w_dt_name), out_specs, in_maps,
                    num_cores=num_cores, trace=trace, sim_only=sim_only)
    if sim_only:
        outs = np.empty((min(num_cores, B), S, H), dtype=np.float32)
        for b in range(outs.shape[0]):
            o = np.asarray(res.cores[b].tensor("outb")).astype(np.float32)
            outs[b] = o.reshape(128, NJT, S).transpose(2, 1, 0).reshape(S, H)
        return outs, res
    outs = np.empty((B, S, H), dtype=np.float32)
    for b in range(B):
        o = np.asarray(res.results[b]["outb"]).astype(np.float32)
        outs[b] = o.reshape(128, NJT, S).transpose(2, 1, 0).reshape(S, H)
    return outs, res


def kernel(**inputs):
    out, _ = run_on_device(inputs)
    return out


# revision 18
# speedup vs baseline: 1.6388x; 1.0037x over previous
"""Trainium2 Bass kernel for nn_MergedConvLiquid (v2).

Model: out = sc + 0.01*(liq - sc) where
  sc  = depthwise causal conv (K=4) over seq,
  liq = per-step gated liquid recurrence with LayerNorm (S sequential steps).

v2 strategy (vs baseline): keep one batch row per core (cores 0-3, with 4-7
duplicating), but restructure the recurrence so the PE weight stream is the
only serial resource and every other engine (DVE / Act / GpSimd) runs off
the critical path:

  - Unnormalized carry c (fp16): LayerNorm's mean subtraction is dropped
    from the state (validated: bounded drift, exact at output via the
    rho/mu histories); the 1/sqrt(var) scale rho is applied lazily.
  - rho(t+1) = one warm-started Newton step from rho(t) toward
    rsqrt(var(t-1) + eps): one-step-lagged LN scale, so the scale for the
    next matvec is ready mid-stream instead of serializing on this step's
    statistics. Error ~ per-step var drift (<6%) -> ~7e-4 on the output.
  - tau MLP runs one step ahead on h(t-1) ("stale tau", error ~3e-5):
    lambda/a for step t+1 are computed during stream t, so the chain
    f->c' never waits on the tau path.
  - Cross-partition sums (tau dot, LN stats) via fp16 ones-matmuls placed
    late in the PE stream so the PE never stalls on them.
  - sigmoid(x) computed as 0.5*(1+tanh(x/2)) so the Act engine only needs
    {tanh, square} (one activation table - no ACT_TABLE_LOAD thrash).
  - conv + blend + output assembly hidden inside the steady loop (column
    t-8 finalized during step t) on spare DVE/GpSimd cycles.
"""

import numpy as np

B, S, H, K = 4, 2048, 1024, 4
DT_, TAU_MIN, TAU_MAX = 0.1, 1.0, 5.0
SCALE = 0.01
LN_EPS = 1e-5
HID = H // 2          # tau hidden width (512)
NJT = H // 128        # 8 j-tiles for H
NHT = HID // 128      # 4 tiles for tau hidden
NKT = H // 128        # 8 k-tiles
NCT = NJT + NHT       # 12 column tiles of Wfull
PRO = 8               # prologue steps (exact rho)


def _build_kernel(n_steps, unroll=8, w_dt_name="float16"):
    from concourse import bass, mybir
    from concourse.engine_type import EngineType

    DT = mybir.dt
    AF = mybir.ActivationFunctionType
    OP = mybir.AluOpType
    AX = mybir.AxisListType
    ds = bass.ds

    assert (n_steps - PRO) % unroll == 0

    def kernel_fn(tc, outs, ins):
        nc = tc.nc
        f32, f16 = DT.float32, DT.float16
        wdt = getattr(DT, w_dt_name)
        SP1 = S + 1

        with tc.tile_pool(name="state", bufs=1) as st, \
             tc.tile_pool(name="scr", bufs=2) as scr, \
             tc.tile_pool(name="psum_zr", bufs=2, space="PSUM") as p_zr, \
             tc.tile_pool(name="psum_zt", bufs=2, space="PSUM") as p_zt, \
             tc.tile_pool(name="psum_sm", bufs=2, space="PSUM") as p_sm, \
             tc.tile_pool(name="psum_x", bufs=2, space="PSUM") as p_x:

            # ---- persistent SBUF state ----
            wsb = st.tile([128, NKT * NCT * 128], wdt)      # W tiles (k,ci)
            w1x = st.tile([128, NKT * NHT * 128], f16)      # tau_w1_x tiles
            xt = st.tile([128, NJT * S], f16)               # x row [p, jt*S+t]
            sc16 = st.tile([128, NJT * S], f16)             # conv result
            outb = st.tile([128, NJT * S], f16)             # final output
            liqs = st.tile([128, NJT * SP1], f16)           # c history (slot t+1)
            xw1s = st.tile([128, NHT * SP1], f16)           # x@tau_w1_x + b1
            rh = st.tile([128, S], f32)                     # rho(t+1) history
            rmuh = st.tile([128, S], f32)                   # -rho(t+1)*mu(t) hist
            hq = [st.tile([128, NJT], f16, name=f"hq{i}") for i in range(2)]
            rho = [st.tile([128, 1], f32, name=f"rho{i}") for i in range(2)]
            lam = [st.tile([128, 1], f32, name=f"lam{i}") for i in range(2)]
            at = [st.tile([128, 1], f32, name=f"at{i}") for i in range(2)]
            stt_ = [st.tile([128, 2], f32, name=f"stt{i}") for i in range(2)]
            st16 = [st.tile([128, 2], f16, name=f"st16_{i}") for i in range(2)]
            ones = st.tile([128, 128], f16)
            w2 = st.tile([128, NHT], f32)
            b2h = st.tile([128, 1], f32)                    # 0.5*b2
            b1 = st.tile([128, NHT], f32)
            cw = st.tile([128, NJT * K], f32)
            lng = st.tile([128, NJT], f32)
            lnb = st.tile([128, NJT], f32)

            nc.sync.dma_start(wsb[:], ins["wq"][:])
            nc.sync.dma_start(w1x[:], ins["w1x"][:])
            nc.sync.dma_start(xt[:], ins["xt"][:])
            nc.sync.dma_start(ones[:], ins["ones"][:])
            nc.sync.dma_start(w2[:], ins["w2"][:])
            nc.sync.dma_start(b2h[:], ins["b2h"][:])
            nc.sync.dma_start(b1[:], ins["b1"][:])
            nc.sync.dma_start(cw[:], ins["cw"][:])
            nc.sync.dma_start(lng[:], ins["lng"][:])
            nc.sync.dma_start(lnb[:], ins["lnb"][:])
            nc.gpsimd.memset(hq[0][:], 0.0)
            nc.gpsimd.memset(rho[0][:], 1.0)
            nc.vector.memset(liqs[:, ds(0, NJT, SP1)], 0.0)
            if n_steps < S:
                nc.gpsimd.memset(outb[:], 0.0)   # test mode: unwritten tail
                nc.gpsimd.memset(rh[:], 0.0)
                nc.gpsimd.memset(rmuh[:], 0.0)
                nc.gpsimd.memset(liqs[:], 0.0)

            # ---- xw1s = cast16(x) @ tau_w1_x + b1 (stride S+1 layout) ----
            TC = 512
            for tci in range(S // TC):
                for hti in range(NHT):
                    px = p_x.tile([128, TC], f32, tag="px")
                    for k in range(NKT):
                        nc.tensor.matmul(
                            px[:],
                            w1x[:, (k * NHT + hti) * 128:(k * NHT + hti) * 128 + 128],
                            xt[:, k * S + tci * TC: k * S + tci * TC + TC],
                            start=(k == 0), stop=(k == NKT - 1))
                    nc.vector.tensor_scalar(
                        xw1s[:, hti * SP1 + tci * TC: hti * SP1 + tci * TC + TC],
                        px[:], b1[:, hti:hti + 1], None, OP.add)
            nc.vector.memset(xw1s[:, ds(S, NHT, SP1)], 0.0)

            # ---- conv precompute: sc16[jt] (DVE for jt 0..4, gpsimd 5..7) ----
            for jt in range(NJT):
                eng = nc.vector
                xs = xt[:, jt * S:(jt + 1) * S]
                scs = sc16[:, jt * S:(jt + 1) * S]
                eng.tensor_scalar(
                    scs, xs, cw[:, jt * K + (K - 1): jt * K + K], None, OP.mult)
                for k in range(K - 1):
                    sh = K - 1 - k
                    eng.scalar_tensor_tensor(
                        scs[:, sh:S], xt[:, jt * S: (jt + 1) * S - sh],
                        cw[:, jt * K + k: jt * K + k + 1], scs[:, sh:S],
                        OP.mult, OP.add)

            def quake_rsqrt(out_ap, v_ap, iters=3):
                """exact-ish rsqrt on DVE (prologue only): quake seed + NR."""
                i32 = DT.int32
                y = scr.tile([128, 1], f32, tag="qk_y")
                tn = scr.tile([128, 1], f32, tag="qk_t")
                nc.vector.tensor_scalar(y[:].bitcast(i32), v_ap.bitcast(i32),
                                        1, None, OP.logical_shift_right)
                nc.vector.tensor_scalar(y[:].bitcast(i32), y[:].bitcast(i32),
                                        -1, 0x5F3759DF, OP.mult, OP.add)
                for _ in range(iters):
                    nc.vector.tensor_tensor(tn[:], y[:], y[:], OP.mult)
                    nc.vector.tensor_tensor(tn[:], tn[:], v_ap, OP.mult)
                    nc.vector.tensor_scalar(tn[:], tn[:], -0.5, 1.5, OP.mult, OP.add)
                    nc.vector.tensor_tensor(y[:], y[:], tn[:], OP.mult)
                nc.vector.tensor_copy(out_ap, y[:])

            def mm(out_ap, ci, k, hq_c, start, stop):
                nc.tensor.matmul(
                    out_ap,
                    wsb[:, (k * NCT + ci) * 128:(k * NCT + ci) * 128 + 128],
                    hq_c[:, k:k + 1],
                    start=start, stop=stop, skip_group_check=True)

            last_u1 = [None, None]

            def step(t, par, exact_rho=False, do_final=True, handoff=False):
                """Emit one step. t may be python int (prologue) or RuntimeValue."""
                hq_c, hq_n = hq[par], hq[1 - par]
                rho_c, rho_n = rho[par], rho[1 - par]
                lam_c, lam_n = lam[par], lam[1 - par]
                a_c, a_n = at[par], at[1 - par]
                st_c, st_p = stt_[par], stt_[1 - par]
                st16_c, st16_p = st16[par], st16[1 - par]

                zr = p_zr.tile([128, NJT], f32, tag="zr")
                zt = p_zt.tile([128, NHT], f32, tag="zt")
                sm = p_sm.tile([128, 4], f32, tag="sm")
                ptau = sm[:, 0:1]
                pst = sm[:, 1:3]

                # === prologue-only: exact rho fix for THIS step (needs stats(t-1))
                if exact_rho:
                    # stats-MM(t-1)
                    nc.tensor.matmul(pst, ones[:], st16_p[:],
                                     start=True, stop=True)
                    ps = scr.tile([128, 2], f32, tag="psx")
                    mneg = scr.tile([128, 1], f32, tag="mnegx")
                    vp = scr.tile([128, 1], f32, tag="vpx")
                    nc.vector.tensor_copy(ps[:], pst)
                    nc.vector.tensor_scalar(mneg[:], ps[:, 0:1], -1.0 / H,
                                            None, OP.mult)
                    nc.vector.scalar_tensor_tensor(vp[:], ps[:, 0:1], mneg[:],
                                                   ps[:, 1:2], OP.mult, OP.add)
                    nc.vector.tensor_scalar(vp[:], vp[:], 1.0 / H, LN_EPS,
                                            OP.mult, OP.add)
                    quake_rsqrt(rho_c[:], vp[:])  # rho(t) exact
                    nc.vector.tensor_copy(rh[:, ds(t - 1, 1)], rho_c[:])
                    nc.vector.tensor_tensor(rmuh[:, ds(t - 1, 1)], rho_c[:],
                                            mneg[:], OP.mult)
                    # a(t) = rho(t) - lam(t)*rho(t)
                    lrx = scr.tile([128, 1], f32, tag="lrx")
                    nc.vector.tensor_tensor(lrx[:], lam_c[:], rho_c[:], OP.mult)
                    nc.vector.tensor_tensor(a_c[:], rho_c[:], lrx[:], OP.subtract)
                    if handoff:
                        # steady-style rho(t+1) = NR1(rho(t) -> var(t-1)+eps)
                        r2 = scr.tile([128, 1], f32, tag="r2x")
                        nc.vector.tensor_tensor(r2[:], rho_c[:], rho_c[:], OP.mult)
                        nc.vector.tensor_tensor(r2[:], r2[:], vp[:], OP.mult)
                        nc.vector.tensor_scalar(r2[:], r2[:], -0.5, 1.5,
                                                OP.mult, OP.add)
                        nc.vector.tensor_tensor(rho_n[:], rho_c[:], r2[:], OP.mult)
                        nc.vector.tensor_copy(rh[:, ds(t, 1)], rho_n[:])

                # === PE: tau block first (z_tau for step t+1)
                for ci in range(NJT, NCT):
                    for k in range(NKT):
                        mm(zt[:, ci - NJT:ci - NJT + 1], ci, k, hq_c,
                           k == 0, k == 7)

                # === early independent DVE work: a*c and x-slice staging
                ac = scr.tile([128, NJT], f32, tag="ac")
                xcur = scr.tile([128, NJT], f32, tag="xcur")
                nc.vector.tensor_scalar(ac[:], hq_c[:], a_c[:], None, OP.mult)
                nc.vector.tensor_copy(xcur[:], xt[:, ds(t, NJT, S)])

                # === stats-MM(t-1) + rho chain (steady only; prologue did exact)
                skip_stats = isinstance(t, int) and t == 0
                if not exact_rho and not skip_stats:
                    nc.tensor.matmul(pst, ones[:], st16_p[:],
                                     start=True, stop=True)
                    ps = scr.tile([128, 2], f32, tag="ps")
                    mneg = scr.tile([128, 1], f32, tag="mneg")
                    vp = scr.tile([128, 1], f32, tag="vp")
                    nc.vector.tensor_copy(ps[:], pst)
                    nc.gpsimd.tensor_scalar(mneg[:], ps[:, 0:1], -1.0 / H,
                                            None, OP.mult)
                    nc.vector.scalar_tensor_tensor(vp[:], ps[:, 0:1], mneg[:],
                                                   ps[:, 1:2], OP.mult, OP.add)
                    nc.gpsimd.tensor_scalar(vp[:], vp[:], 1.0 / H, LN_EPS,
                                            OP.mult, OP.add)
                    # NR1: rho_n = rho_c*(1.5 - 0.5*vp*rho_c^2)
                    r2 = scr.tile([128, 1], f32, tag="r2")
                    nc.gpsimd.tensor_tensor(r2[:], rho_c[:], rho_c[:], OP.mult)
                    nc.gpsimd.tensor_tensor(r2[:], r2[:], vp[:], OP.mult)
                    nc.gpsimd.tensor_scalar(r2[:], r2[:], -0.5, 1.5,
                                            OP.mult, OP.add)
                    nc.gpsimd.tensor_tensor(rho_n[:], rho_c[:], r2[:], OP.mult)
                    nc.gpsimd.tensor_copy(rh[:, ds(t, 1)], rho_n[:])
                    nc.gpsimd.tensor_tensor(rmuh[:, ds(t - 1, 1)], rho_c[:],
                                            mneg[:], OP.mult)

                # === tau chain for step t+1 (stale h)
                u_tau = scr.tile([128, NHT], f32, tag="ut")
                tu = scr.tile([128, NHT], f16, tag="tu")
                junk = scr.tile([128, NHT], f32, tag="junk")
                taud = scr.tile([128, 1], f16, tag="taud")
                nc.vector.scalar_tensor_tensor(
                    u_tau[:], zt[:], rho_c[:], xw1s[:, ds(t + 1, NHT, SP1)],
                    OP.mult, OP.add)
                nc.scalar.activation(tu[:], u_tau[:], AF.Tanh)
                nc.vector.tensor_tensor(junk[:], tu[:], w2[:], OP.mult)
                with nc.allow_low_precision(reason="4-elem tau dot, fp16 ample"):
                    nc.vector.tensor_reduce(taud[:], junk[:], AX.X, OP.add)

                # === PE: rec block (ci 0..7)
                for ci in range(8):
                    for k in range(NKT):
                        mm(zr[:, ci:ci + 1], ci, k, hq_c, k == 0, k == 7)

                # === taud ones-MM
                nc.tensor.matmul(ptau, ones[:], taud[:], start=True, stop=True)

                # === DVE chain: u1 -> f -> c'  (ac precomputed)
                u1 = scr.tile([128, NJT], f32, tag="u1")
                last_u1[0] = u1
                ff = scr.tile([128, NJT], f32, tag="ff")
                nc.vector.scalar_tensor_tensor(
                    u1[:], zr[:], rho_c[:], xcur[:], OP.mult, OP.add)
                nc.scalar.activation(ff[:], u1[:], AF.Tanh)
                nc.vector.scalar_tensor_tensor(
                    hq_n[:], ff[:], lam_c[:], ac[:], OP.mult, OP.add,
                    accum_out=st_c[:, 0:1])
                nc.vector.tensor_copy(liqs[:, ds(t + 1, NJT, SP1)], hq_n[:])
                sq = scr.tile([128, NJT], f16, tag="sq")
                nc.scalar.activation(sq[:], hq_n[:], AF.Square,
                                     accum_out=st_c[:, 1:2])
                nc.vector.tensor_copy(st16_c[:], st_c[:, 0:2])

                # === sigmoid chain -> lam(t+1), a(t+1)
                th = scr.tile([128, 1], f32, tag="th")
                tauv = scr.tile([128, 1], f32, tag="tauv")
                itau = scr.tile([128, 1], f32, tag="itau")
                lr = scr.tile([128, 1], f32, tag="lr")
                nc.scalar.activation(th[:], ptau, AF.Tanh,
                                     bias=b2h[:], scale=0.5)
                nc.gpsimd.tensor_scalar(tauv[:], th[:], 2.0, 3.0, OP.mult, OP.add)
                nc.vector.reciprocal(itau[:], tauv[:])
                nc.gpsimd.tensor_scalar(lam_n[:], itau[:], DT_, None, OP.mult)
                if not handoff and not exact_rho and not skip_stats:
                    nc.gpsimd.tensor_tensor(lr[:], lam_n[:], rho_n[:], OP.mult)
                    nc.gpsimd.tensor_tensor(a_n[:], rho_n[:], lr[:], OP.subtract)
                elif handoff:
                    nc.vector.tensor_tensor(lr[:], lam_n[:], rho_n[:], OP.mult)
                    nc.vector.tensor_tensor(a_n[:], rho_n[:], lr[:], OP.subtract)
                # (plain prologue: a(t+1) computed by next prologue step's
                #  exact-rho fix; lam_n is stored for it.)

                # === hidden final for column t' = t - PRO
                if do_final:
                    tp = t - PRO
                    lq8 = scr.tile([128, NJT], f32, tag="lq8")
                    m1 = scr.tile([128, NJT], f32, tag="m1")
                    o1 = scr.tile([128, NJT], f32, tag="o1")
                    nc.vector.tensor_scalar(
                        lq8[:], liqs[:, ds(tp + 1, NJT, SP1)],
                        rh[:, ds(tp, 1)], rmuh[:, ds(tp, 1)], OP.mult, OP.add)
                    nc.gpsimd.tensor_tensor(m1[:], lq8[:], lng[:], OP.mult)
                    nc.gpsimd.tensor_tensor(m1[:], m1[:], lnb[:], OP.add)
                    nc.gpsimd.tensor_scalar(o1[:], m1[:], SCALE, None, OP.mult)
                    nc.vector.scalar_tensor_tensor(
                        outb[:, ds(tp, NJT, S)], sc16[:, ds(tp, NJT, S)],
                        1.0 - SCALE, o1[:], OP.mult, OP.add)

            # ---- bootstrap: lam(0), a(0) from xw1s col 0 (z_tau(-1) = 0) ----
            tu0 = scr.tile([128, NHT], f16, tag="tu0")
            junk0 = scr.tile([128, NHT], f32, tag="junk0")
            taud0 = scr.tile([128, 1], f16, tag="taud0")
            sm0 = p_sm.tile([128, 4], f32, tag="sm")
            pt0 = sm0[:, 0:1]
            th0 = scr.tile([128, 1], f32, tag="th0")
            tv0 = scr.tile([128, 1], f32, tag="tv0")
            it0 = scr.tile([128, 1], f32, tag="it0")
            nc.scalar.activation(tu0[:], xw1s[:, ds(0, NHT, SP1)], AF.Tanh)
            nc.vector.tensor_tensor(junk0[:], tu0[:], w2[:], OP.mult)
            with nc.allow_low_precision(reason="4-elem tau dot, fp16 ample"):
                nc.vector.tensor_reduce(taud0[:], junk0[:], AX.X, OP.add)
            nc.tensor.matmul(pt0, ones[:], taud0[:], start=True, stop=True)
            nc.scalar.activation(th0[:], pt0, AF.Tanh, bias=b2h[:], scale=0.5)
            nc.vector.tensor_scalar(tv0[:], th0[:], 2.0, 3.0, OP.mult, OP.add)
            nc.vector.reciprocal(it0[:], tv0[:])
            nc.vector.tensor_scalar(lam[0][:], it0[:], DT_, None, OP.mult)
            nc.vector.tensor_scalar(at[0][:], lam[0][:], -1.0, 1.0,
                                    OP.mult, OP.add)

            # ---- prologue steps 0..PRO-1 ----
            for t in range(PRO):
                step(t, t & 1, exact_rho=(t >= 1), do_final=False,
                     handoff=(t == PRO - 1))

            # ---- steady loop ----
            with tc.For_i(PRO, n_steps, unroll,
                          hint_engines=(EngineType.PE, EngineType.DVE,
                                        EngineType.Activation,
                                        EngineType.Pool)) as iv:
                for u in range(unroll):
                    step(iv + u, (PRO + u) & 1, do_final=True)

            # ---- tail ----
            # stats(n-1) -> rmuh[n-1] = -rho(n)*mu(n-1)
            par_last = (n_steps - 1) & 1          # parity used by last step
            smT = p_sm.tile([128, 4], f32, tag="sm")
            pstT = smT[:, 1:3]
            nc.tensor.matmul(pstT, ones[:], st16[par_last][:],
                             start=True, stop=True)
            mnegT = scr.tile([128, 1], f32, tag="mnegT")
            nc.vector.tensor_scalar(mnegT[:], smT[:, 1:2], -1.0 / H,
                                    None, OP.mult)
            nc.vector.tensor_tensor(rmuh[:, ds(n_steps - 1, 1)],
                                    rho[1 - par_last][:], mnegT[:], OP.mult)
            # final columns n_steps-PRO .. n_steps-1
            for tp in range(n_steps - PRO, n_steps):
                lq8 = scr.tile([128, NJT], f32, tag="lq8T")
                m1 = scr.tile([128, NJT], f32, tag="m1T")
                o1 = scr.tile([128, NJT], f32, tag="o1T")
                nc.vector.tensor_scalar(
                    lq8[:], liqs[:, ds(tp + 1, NJT, SP1)],
                    rh[:, ds(tp, 1)], rmuh[:, ds(tp, 1)], OP.mult, OP.add)
                nc.vector.tensor_tensor(m1[:], lq8[:], lng[:], OP.mult)
                nc.vector.tensor_tensor(m1[:], m1[:], lnb[:], OP.add)
                nc.vector.tensor_scalar(o1[:], m1[:], SCALE, None, OP.mult)
                nc.vector.scalar_tensor_tensor(
                    outb[:, ds(tp, NJT, S)], sc16[:, ds(tp, NJT, S)],
                    1.0 - SCALE, o1[:], OP.mult, OP.add)

            nc.sync.dma_start(outs["outb"], outb[:])
            if "dbg_zr" in outs:
                zdump = st.tile([128, NJT], f32, name="zdump")
                nc.vector.tensor_copy(zdump[:], last_u1[0][:])
                nc.sync.dma_start(outs["dbg_zr"], zdump[:])
            if "dbg_rh" in outs:
                nc.sync.dma_start(outs["dbg_rh"], rh[:])
                nc.sync.dma_start(outs["dbg_rmuh"], rmuh[:])
                nc.sync.dma_start(outs["dbg_liqs"], liqs[:])

    return kernel_fn


def _prep_in_maps(hidden_states, conv_w, W_rec, tau_w1, tau_b1, tau_w2, tau_b2,
                  ln_g, ln_b, w_dt_name="float16"):
    """Host-side staging: per-core input dict (core c gets batch row c%4)."""
    if w_dt_name == "float16":
        np_wdt = np.float16
    else:
        import ml_dtypes
        np_wdt = ml_dtypes.float8_e4m3fn
    x = np.asarray(hidden_states, dtype=np.float32)
    Wfull = np.concatenate([np.asarray(W_rec).T, np.asarray(tau_w1)[H:]], axis=1)
    # [kk, kt, ct, jj]
    wq = Wfull.reshape(NKT, 128, NCT, 128).transpose(1, 0, 2, 3)
    wq = np.ascontiguousarray(wq).astype(np_wdt).reshape(128, NKT * NCT * 128)
    w1xh = np.asarray(tau_w1)[:H]  # [H, HID]
    w1x = w1xh.reshape(NKT, 128, NHT, 128).transpose(1, 0, 2, 3)
    w1x = np.ascontiguousarray(w1x, dtype=np.float16).reshape(128, NKT * NHT * 128)
    w2 = np.ascontiguousarray(
        np.asarray(tau_w2)[:, 0].reshape(NHT, 128).T, dtype=np.float32)
    b1 = np.ascontiguousarray(
        np.asarray(tau_b1).reshape(NHT, 128).T, dtype=np.float32)
    b2h = np.full((128, 1), 0.5 * float(np.asarray(tau_b2)[0]), dtype=np.float32)
    cw = np.ascontiguousarray(
        np.asarray(conv_w).reshape(NJT, 128, K).transpose(1, 0, 2),
        dtype=np.float32).reshape(128, NJT * K)
    lng = np.ascontiguousarray(
        np.asarray(ln_g).reshape(NJT, 128).T, dtype=np.float32)
    lnb = np.ascontiguousarray(
        np.asarray(ln_b).reshape(NJT, 128).T, dtype=np.float32)
    ones = np.ones((128, 128), dtype=np.float16)

    shared = dict(wq=wq, w1x=w1x, w2=w2, b1=b1, b2h=b2h, cw=cw, lng=lng,
                  lnb=lnb, ones=ones)
    in_maps = []
    for c in range(8):
        b = c % B
        xtb = np.ascontiguousarray(
            x[b].T.reshape(NJT, 128, S).transpose(1, 0, 2),
            dtype=np.float16).reshape(128, NJT * S)
        m = dict(shared)
        m["xt"] = xtb
        in_maps.append(m)
    return in_maps


def _in_specs(w_dt_name="float16"):
    from concourse import mybir
    DT = mybir.dt
    wdt = getattr(DT, w_dt_name)
    return {
        "wq": ((128, NKT * NCT * 128), wdt),
        "w1x": ((128, NKT * NHT * 128), DT.float16),
        "xt": ((128, NJT * S), DT.float16),
        "w2": ((128, NHT), DT.float32),
        "b1": ((128, NHT), DT.float32),
        "b2h": ((128, 1), DT.float32),
        "cw": ((128, NJT * K), DT.float32),
        "lng": ((128, NJT), DT.float32),
        "lnb": ((128, NJT), DT.float32),
        "ones": ((128, 128), DT.float16),
    }


def _run_spmd(kernel_fn, in_specs, out_specs, in_maps, num_cores=8, trace=False,
              sim_only=False):
    from concourse import bacc, tile
    from concourse.bass_interp import MultiCoreSim

    nc = bacc.Bacc(
        "TRN2",
        target_bir_lowering=False,
        debug=False,
        enable_asserts=True,
        num_devices=num_cores,
    )
    in_tiles = {
        name: nc.dram_tensor(name, list(shape), dt, kind="ExternalInput").ap()
        for name, (shape, dt) in in_specs.items()
    }
    out_tiles = {
        name: nc.dram_tensor(name, list(shape), dt, kind="ExternalOutput").ap()
        for name, (shape, dt) in out_specs.items()
    }
    with tile.TileContext(nc, trace_sim=True) as tc:
        kernel_fn(tc, out_tiles, in_tiles)
    nc.compile()

    sim = MultiCoreSim(nc, num_cores=num_cores, trace=True)
    for i, core in sim.cores.items():
        for name, arr in in_maps[i].items():
            core.tensor(name)[:] = arr
    if sim_only:
        sim.simulate()
        return sim
    return sim.run_on_hw_raw(trace=trace)


def run_on_device(inputs, n_steps=S, unroll=8, trace=False, w_dt_name="float16",
                  sim_only=False, num_cores=8):
    from concourse import mybir
    DT = mybir.dt
    in_maps = _prep_in_maps(**inputs, w_dt_name=w_dt_name)[:num_cores]
    kernel_fn = _build_kernel(n_steps, unroll=unroll, w_dt_name=w_dt_name)
    out_specs = {"outb": ((128, NJT * S), DT.float16)}
    if sim_only:
        out_specs["dbg_zr"] = ((128, NJT), DT.float32)
        out_specs["dbg_rh"] = ((128, S), DT.float32)
        out_specs["dbg_rmuh"] = ((128, S), DT.float32)
        out_specs["dbg_liqs"] = ((128, NJT * (S + 1)), DT.float16)
    res = _run_spmd(kernel_fn, _in_specs(w_dt_name), out_specs, in_maps,
                    num_cores=num_cores, trace=trace, sim_only=sim_only)
    if sim_only:
        outs = np.empty((min(num_cores, B), S, H), dtype=np.float32)
        for b in range(outs.shape[0]):
            o = np.asarray(res.cores[b].tensor("outb")).astype(np.float32)
            outs[b] = o.reshape(128, NJT, S).transpose(2, 1, 0).reshape(S, H)
        return outs, res
    outs = np.empty((B, S, H), dtype=np.float32)
    for b in range(B):
        o = np.asarray(res.results[b]["outb"]).astype(np.float32)
        outs[b] = o.reshape(128, NJT, S).transpose(2, 1, 0).reshape(S, H)
    return outs, res


def kernel(**inputs):
    out, _ = run_on_device(inputs)
    return out


# revision 20
# speedup vs baseline: 1.9205x; 1.1719x over previous
"""Trainium2 Bass kernel for nn_MergedConvLiquid (v2).

Model: out = sc + 0.01*(liq - sc) where
  sc  = depthwise causal conv (K=4) over seq,
  liq = per-step gated liquid recurrence with LayerNorm (S sequential steps).

v2 strategy (vs baseline): keep one batch row per core (cores 0-3, with 4-7
duplicating), but restructure the recurrence so the PE weight stream is the
only serial resource and every other engine (DVE / Act / GpSimd) runs off
the critical path:

  - Unnormalized carry c (fp16): LayerNorm's mean subtraction is dropped
    from the state (validated: bounded drift, exact at output via the
    rho/mu histories); the 1/sqrt(var) scale rho is applied lazily.
  - rho(t+1) = one warm-started Newton step from rho(t) toward
    rsqrt(var(t-1) + eps): one-step-lagged LN scale, so the scale for the
    next matvec is ready mid-stream instead of serializing on this step's
    statistics. Error ~ per-step var drift (<6%) -> ~7e-4 on the output.
  - tau MLP runs one step ahead on h(t-1) ("stale tau", error ~3e-5):
    lambda/a for step t+1 are computed during stream t, so the chain
    f->c' never waits on the tau path.
  - Cross-partition sums (tau dot, LN stats) via fp16 ones-matmuls placed
    late in the PE stream so the PE never stalls on them.
  - sigmoid(x) computed as 0.5*(1+tanh(x/2)) so the Act engine only needs
    {tanh, square} (one activation table - no ACT_TABLE_LOAD thrash).
  - conv + blend + output assembly hidden inside the steady loop (column
    t-8 finalized during step t) on spare DVE/GpSimd cycles.
"""

import numpy as np

B, S, H, K = 4, 2048, 1024, 4
DT_, TAU_MIN, TAU_MAX = 0.1, 1.0, 5.0
SCALE = 0.01
LN_EPS = 1e-5
HID = H // 2          # tau hidden width (512)
NJT = H // 128        # 8 j-tiles for H
NHT = HID // 128      # 4 tiles for tau hidden
NKT = H // 128        # 8 k-tiles
NCT = NJT + NHT       # 12 column tiles of Wfull
PRO = 16              # prologue steps (exact rho)


def _build_kernel(n_steps, unroll=16, w_dt_name="float16"):
    from concourse import bass, mybir
    from concourse.engine_type import EngineType

    DT = mybir.dt
    AF = mybir.ActivationFunctionType
    OP = mybir.AluOpType
    AX = mybir.AxisListType
    ds = bass.ds

    assert (n_steps - PRO) % unroll == 0

    def kernel_fn(tc, outs, ins):
        nc = tc.nc
        f32, f16 = DT.float32, DT.float16
        wdt = getattr(DT, w_dt_name)
        SP1 = S + 1

        with tc.tile_pool(name="state", bufs=1) as st, \
             tc.tile_pool(name="scr", bufs=2) as scr, \
             tc.tile_pool(name="psum_zr", bufs=2, space="PSUM") as p_zr, \
             tc.tile_pool(name="psum_zt", bufs=2, space="PSUM") as p_zt, \
             tc.tile_pool(name="psum_sm", bufs=2, space="PSUM") as p_sm, \
             tc.tile_pool(name="psum_x", bufs=2, space="PSUM") as p_x:

            # ---- persistent SBUF state ----
            wsb = st.tile([128, NKT * NCT * 128], wdt)      # W tiles (k,ci)
            w1x = st.tile([128, NKT * NHT * 128], f16)      # tau_w1_x tiles
            xt = st.tile([128, NJT * S], f16)               # x row [p, jt*S+t]
            sc16 = st.tile([128, NJT * S], f16)             # conv result
            outb = st.tile([128, NJT * S], f16)             # final output
            liqs = st.tile([128, NJT * SP1], f16)           # c history (slot t+1)
            xw1s = st.tile([128, NHT * SP1], f16)           # x@tau_w1_x + b1
            rh = st.tile([128, S], f32)                     # rho(t+1) history
            rmuh = st.tile([128, S], f32)                   # -rho(t+1)*mu(t) hist
            hq = [st.tile([128, NJT], f16, name=f"hq{i}") for i in range(2)]
            rho = [st.tile([128, 1], f32, name=f"rho{i}") for i in range(2)]
            lam = [st.tile([128, 1], f32, name=f"lam{i}") for i in range(2)]
            at = [st.tile([128, 1], f32, name=f"at{i}") for i in range(2)]
            stt_ = [st.tile([128, 2], f32, name=f"stt{i}") for i in range(2)]
            st16 = [st.tile([128, 2], f16, name=f"st16_{i}") for i in range(2)]
            ones = st.tile([128, 128], f16)
            w2 = st.tile([128, NHT], f32)
            b2h = st.tile([128, 1], f32)                    # 0.5*b2
            b1 = st.tile([128, NHT], f32)
            cw = st.tile([128, NJT * K], f32)
            lng = st.tile([128, NJT], f32)
            lnb = st.tile([128, NJT], f32)

            nc.sync.dma_start(wsb[:], ins["wq"][:])
            nc.sync.dma_start(w1x[:], ins["w1x"][:])
            nc.sync.dma_start(xt[:], ins["xt"][:])
            nc.sync.dma_start(ones[:], ins["ones"][:])
            nc.sync.dma_start(w2[:], ins["w2"][:])
            nc.sync.dma_start(b2h[:], ins["b2h"][:])
            nc.sync.dma_start(b1[:], ins["b1"][:])
            nc.sync.dma_start(cw[:], ins["cw"][:])
            nc.sync.dma_start(lng[:], ins["lng"][:])
            nc.sync.dma_start(lnb[:], ins["lnb"][:])
            nc.gpsimd.memset(hq[0][:], 0.0)
            nc.gpsimd.memset(hq[1][:], 0.0)
            nc.gpsimd.memset(rho[0][:], 1.0)
            nc.vector.memset(liqs[:, ds(0, NJT, SP1)], 0.0)
            if n_steps < S:
                nc.gpsimd.memset(outb[:], 0.0)   # test mode: unwritten tail
                nc.gpsimd.memset(rh[:], 0.0)
                nc.gpsimd.memset(rmuh[:], 0.0)
                nc.gpsimd.memset(liqs[:], 0.0)

            # ---- xw1s = cast16(x) @ tau_w1_x + b1 (stride S+1 layout) ----
            TC = 512
            for tci in range(S // TC):
                for hti in range(NHT):
                    px = p_x.tile([128, TC], f32, tag="px")
                    for k in range(NKT):
                        nc.tensor.matmul(
                            px[:],
                            w1x[:, (k * NHT + hti) * 128:(k * NHT + hti) * 128 + 128],
                            xt[:, k * S + tci * TC: k * S + tci * TC + TC],
                            start=(k == 0), stop=(k == NKT - 1))
                    nc.vector.tensor_scalar(
                        xw1s[:, hti * SP1 + tci * TC: hti * SP1 + tci * TC + TC],
                        px[:], b1[:, hti:hti + 1], None, OP.add)
            nc.vector.memset(xw1s[:, ds(S, NHT, SP1)], 0.0)

            # ---- conv precompute: sc16[jt] (DVE for jt 0..4, gpsimd 5..7) ----
            for jt in range(NJT):
                eng = nc.vector
                xs = xt[:, jt * S:(jt + 1) * S]
                scs = sc16[:, jt * S:(jt + 1) * S]
                eng.tensor_scalar(
                    scs, xs, cw[:, jt * K + (K - 1): jt * K + K], None, OP.mult)
                for k in range(K - 1):
                    sh = K - 1 - k
                    eng.scalar_tensor_tensor(
                        scs[:, sh:S], xt[:, jt * S: (jt + 1) * S - sh],
                        cw[:, jt * K + k: jt * K + k + 1], scs[:, sh:S],
                        OP.mult, OP.add)

            def quake_rsqrt(out_ap, v_ap, iters=3):
                """exact-ish rsqrt on DVE (prologue only): quake seed + NR."""
                i32 = DT.int32
                y = scr.tile([128, 1], f32, tag="qk_y")
                tn = scr.tile([128, 1], f32, tag="qk_t")
                nc.vector.tensor_scalar(y[:].bitcast(i32), v_ap.bitcast(i32),
                                        1, None, OP.logical_shift_right)
                nc.vector.tensor_scalar(y[:].bitcast(i32), y[:].bitcast(i32),
                                        -1, 0x5F3759DF, OP.mult, OP.add)
                for _ in range(iters):
                    nc.vector.tensor_tensor(tn[:], y[:], y[:], OP.mult)
                    nc.vector.tensor_tensor(tn[:], tn[:], v_ap, OP.mult)
                    nc.vector.tensor_scalar(tn[:], tn[:], -0.5, 1.5, OP.mult, OP.add)
                    nc.vector.tensor_tensor(y[:], y[:], tn[:], OP.mult)
                nc.vector.tensor_copy(out_ap, y[:])

            def mm(out_ap, ci, k, hq_c, start, stop):
                nc.tensor.matmul(
                    out_ap,
                    wsb[:, (k * NCT + ci) * 128:(k * NCT + ci) * 128 + 128],
                    hq_c[:, k:k + 1],
                    start=start, stop=stop, skip_group_check=True)

            last_u1 = [None, None]

            def step(t, par, exact_rho=False, do_final=True, handoff=False):
                """Emit one step. t may be python int (prologue) or RuntimeValue."""
                hq_c, hq_n = hq[par], hq[1 - par]
                rho_c, rho_n = rho[par], rho[1 - par]
                lam_c, lam_n = lam[par], lam[1 - par]
                a_c, a_n = at[par], at[1 - par]
                st_c, st_p = stt_[par], stt_[1 - par]
                st16_c, st16_p = st16[par], st16[1 - par]

                zr = p_zr.tile([128, NJT], f32, tag="zr")
                zt = p_zt.tile([128, NHT], f32, tag="zt")
                sm = p_sm.tile([128, 4], f32, tag="sm")
                ptau = sm[:, 0:1]
                pst = sm[:, 1:3]

                # === prologue-only: exact rho fix for THIS step (needs stats(t-1))
                if exact_rho:
                    # stats-MM(t-1)
                    nc.tensor.matmul(pst, ones[:], st16_p[:],
                                     start=True, stop=True)
                    ps = scr.tile([128, 2], f32, tag="psx")
                    mneg = scr.tile([128, 1], f32, tag="mnegx")
                    vp = scr.tile([128, 1], f32, tag="vpx")
                    nc.vector.tensor_copy(ps[:], pst)
                    nc.vector.tensor_scalar(mneg[:], ps[:, 0:1], -1.0 / H,
                                            None, OP.mult)
                    nc.vector.scalar_tensor_tensor(vp[:], ps[:, 0:1], mneg[:],
                                                   ps[:, 1:2], OP.mult, OP.add)
                    nc.vector.tensor_scalar(vp[:], vp[:], 1.0 / H, LN_EPS,
                                            OP.mult, OP.add)
                    quake_rsqrt(rho_c[:], vp[:])  # rho(t) exact
                    nc.vector.tensor_copy(rh[:, ds(t - 1, 1)], rho_c[:])
                    nc.vector.tensor_tensor(rmuh[:, ds(t - 1, 1)], rho_c[:],
                                            mneg[:], OP.mult)
                    # a(t) = rho(t) - lam(t)*rho(t)
                    lrx = scr.tile([128, 1], f32, tag="lrx")
                    nc.vector.tensor_tensor(lrx[:], lam_c[:], rho_c[:], OP.mult)
                    nc.vector.tensor_tensor(a_c[:], rho_c[:], lrx[:], OP.subtract)
                    if handoff:
                        # steady-style rho(t+1) = NR1(rho(t) -> var(t-1)+eps)
                        r2 = scr.tile([128, 1], f32, tag="r2x")
                        nc.vector.tensor_tensor(r2[:], rho_c[:], rho_c[:], OP.mult)
                        nc.vector.tensor_tensor(r2[:], r2[:], vp[:], OP.mult)
                        nc.vector.tensor_scalar(r2[:], r2[:], -0.5, 1.5,
                                                OP.mult, OP.add)
                        nc.vector.tensor_tensor(rho_n[:], rho_c[:], r2[:], OP.mult)
                        nc.vector.tensor_copy(rh[:, ds(t, 1)], rho_n[:])

                # === PE: tau block first (z_tau for step t+1, h(t-2) stale)
                for ci in range(NJT, NCT):
                    for k in range(NKT):
                        mm(zt[:, ci - NJT:ci - NJT + 1], ci, k, hq_n,
                           k == 0, k == 7)

                # === early independent DVE work: a*c and x-slice staging
                ac = scr.tile([128, NJT], f32, tag="ac")
                xcur = scr.tile([128, NJT], f32, tag="xcur")
                nc.vector.tensor_scalar(ac[:], hq_c[:], a_c[:], None, OP.mult)
                nc.vector.tensor_copy(xcur[:], xt[:, ds(t, NJT, S)])

                # === stats-MM(t-1) on PE (chain ops emitted later)
                skip_stats = isinstance(t, int) and t == 0
                if not exact_rho and not skip_stats:
                    nc.tensor.matmul(pst, ones[:], st16_p[:],
                                     start=True, stop=True)

                # === tau chain for step t+1 (stale h)
                u_tau = scr.tile([128, NHT], f32, tag="ut")
                tu = scr.tile([128, NHT], f16, tag="tu")
                junk = scr.tile([128, NHT], f32, tag="junk")
                taud = scr.tile([128, 1], f16, tag="taud")
                nc.vector.scalar_tensor_tensor(
                    u_tau[:], zt[:], rho_c[:], xw1s[:, ds(t + 1, NHT, SP1)],
                    OP.mult, OP.add)
                nc.scalar.activation(tu[:], u_tau[:], AF.Tanh)
                nc.vector.tensor_tensor(junk[:], tu[:], w2[:], OP.mult)
                with nc.allow_low_precision(reason="4-elem tau dot, fp16 ample"):
                    nc.vector.tensor_reduce(taud[:], junk[:], AX.X, OP.add)

                # === PE: rec block (ci 0..7)
                for ci in range(8):
                    for k in range(NKT):
                        mm(zr[:, ci:ci + 1], ci, k, hq_c, k == 0, k == 7)

                # === taud ones-MM
                nc.tensor.matmul(ptau, ones[:], taud[:], start=True, stop=True)

                # === DVE chain: u1 -> f -> c'  (ac precomputed)
                u1 = scr.tile([128, NJT], f32, tag="u1")
                last_u1[0] = u1
                ff = scr.tile([128, NJT], f32, tag="ff")
                nc.vector.scalar_tensor_tensor(
                    u1[:], zr[:], rho_c[:], xcur[:], OP.mult, OP.add)
                nc.scalar.activation(ff[:], u1[:], AF.Tanh)
                nc.vector.scalar_tensor_tensor(
                    hq_n[:], ff[:], lam_c[:], ac[:], OP.mult, OP.add,
                    accum_out=st_c[:, 0:1])
                nc.vector.tensor_copy(liqs[:, ds(t + 1, NJT, SP1)], hq_n[:])
                sq = scr.tile([128, NJT], f16, tag="sq")
                nc.scalar.activation(sq[:], hq_n[:], AF.Square,
                                     accum_out=st_c[:, 1:2])
                nc.vector.tensor_copy(st16_c[:], st_c[:, 0:2])

                # === stats chain (rho for t+1; consumed next step - slack ok)
                if not exact_rho and not skip_stats:
                    ps = scr.tile([128, 2], f32, tag="ps")
                    mneg = scr.tile([128, 1], f32, tag="mneg")
                    vp = scr.tile([128, 1], f32, tag="vp")
                    nc.vector.tensor_copy(ps[:], pst)
                    nc.gpsimd.tensor_scalar(mneg[:], ps[:, 0:1], -1.0 / H,
                                            None, OP.mult)
                    nc.vector.scalar_tensor_tensor(vp[:], ps[:, 0:1], mneg[:],
                                                   ps[:, 1:2], OP.mult, OP.add)
                    nc.gpsimd.tensor_scalar(vp[:], vp[:], 1.0 / H, LN_EPS,
                                            OP.mult, OP.add)
                    r2 = scr.tile([128, 1], f32, tag="r2")
                    nc.gpsimd.tensor_tensor(r2[:], rho_c[:], rho_c[:], OP.mult)
                    nc.gpsimd.tensor_tensor(r2[:], r2[:], vp[:], OP.mult)
                    nc.gpsimd.tensor_scalar(r2[:], r2[:], -0.5, 1.5,
                                            OP.mult, OP.add)
                    nc.gpsimd.tensor_tensor(rho_n[:], rho_c[:], r2[:], OP.mult)
                    nc.gpsimd.tensor_copy(rh[:, ds(t, 1)], rho_n[:])
                    nc.gpsimd.tensor_tensor(rmuh[:, ds(t - 1, 1)], rho_c[:],
                                            mneg[:], OP.mult)

                # === sigmoid chain -> lam(t+1), a(t+1)
                th = scr.tile([128, 1], f32, tag="th")
                tauv = scr.tile([128, 1], f32, tag="tauv")
                itau = scr.tile([128, 1], f32, tag="itau")
                lr = scr.tile([128, 1], f32, tag="lr")
                nc.scalar.activation(th[:], ptau, AF.Tanh,
                                     bias=b2h[:], scale=0.5)
                nc.gpsimd.tensor_scalar(tauv[:], th[:], 2.0, 3.0, OP.mult, OP.add)
                nc.vector.reciprocal(itau[:], tauv[:])
                nc.gpsimd.tensor_scalar(lam_n[:], itau[:], DT_, None, OP.mult)
                if not handoff and not exact_rho and not skip_stats:
                    nc.gpsimd.tensor_tensor(lr[:], lam_n[:], rho_n[:], OP.mult)
                    nc.gpsimd.tensor_tensor(a_n[:], rho_n[:], lr[:], OP.subtract)
                elif handoff:
                    nc.vector.tensor_tensor(lr[:], lam_n[:], rho_n[:], OP.mult)
                    nc.vector.tensor_tensor(a_n[:], rho_n[:], lr[:], OP.subtract)
                # (plain prologue: a(t+1) computed by next prologue step's
                #  exact-rho fix; lam_n is stored for it.)

                # === hidden final for column t' = t - PRO
                if do_final:
                    tp = t - PRO
                    lq8 = scr.tile([128, NJT], f32, tag="lq8")
                    m1 = scr.tile([128, NJT], f32, tag="m1")
                    o1 = scr.tile([128, NJT], f32, tag="o1")
                    nc.vector.tensor_scalar(
                        lq8[:], liqs[:, ds(tp + 1, NJT, SP1)],
                        rh[:, ds(tp, 1)], rmuh[:, ds(tp, 1)], OP.mult, OP.add)
                    nc.gpsimd.tensor_tensor(m1[:], lq8[:], lng[:], OP.mult)
                    nc.gpsimd.tensor_tensor(m1[:], m1[:], lnb[:], OP.add)
                    nc.gpsimd.tensor_scalar(o1[:], m1[:], SCALE, None, OP.mult)
                    nc.vector.scalar_tensor_tensor(
                        outb[:, ds(tp, NJT, S)], sc16[:, ds(tp, NJT, S)],
                        1.0 - SCALE, o1[:], OP.mult, OP.add)

            # ---- bootstrap: lam(0), a(0) from xw1s col 0 (z_tau(-1) = 0) ----
            tu0 = scr.tile([128, NHT], f16, tag="tu0")
            junk0 = scr.tile([128, NHT], f32, tag="junk0")
            taud0 = scr.tile([128, 1], f16, tag="taud0")
            sm0 = p_sm.tile([128, 4], f32, tag="sm")
            pt0 = sm0[:, 0:1]
            th0 = scr.tile([128, 1], f32, tag="th0")
            tv0 = scr.tile([128, 1], f32, tag="tv0")
            it0 = scr.tile([128, 1], f32, tag="it0")
            nc.scalar.activation(tu0[:], xw1s[:, ds(0, NHT, SP1)], AF.Tanh)
            nc.vector.tensor_tensor(junk0[:], tu0[:], w2[:], OP.mult)
            with nc.allow_low_precision(reason="4-elem tau dot, fp16 ample"):
                nc.vector.tensor_reduce(taud0[:], junk0[:], AX.X, OP.add)
            nc.tensor.matmul(pt0, ones[:], taud0[:], start=True, stop=True)
            nc.scalar.activation(th0[:], pt0, AF.Tanh, bias=b2h[:], scale=0.5)
            nc.vector.tensor_scalar(tv0[:], th0[:], 2.0, 3.0, OP.mult, OP.add)
            nc.vector.reciprocal(it0[:], tv0[:])
            nc.vector.tensor_scalar(lam[0][:], it0[:], DT_, None, OP.mult)
            nc.vector.tensor_scalar(at[0][:], lam[0][:], -1.0, 1.0,
                                    OP.mult, OP.add)

            # ---- prologue steps 0..PRO-1 ----
            for t in range(PRO):
                step(t, t & 1, exact_rho=(t >= 1), do_final=False,
                     handoff=(t == PRO - 1))

            # ---- steady loop ----
            with tc.For_i(PRO, n_steps, unroll,
                          hint_engines=(EngineType.PE, EngineType.DVE,
                                        EngineType.Activation,
                                        EngineType.Pool)) as iv:
                for u in range(unroll):
                    step(iv + u, (PRO + u) & 1, do_final=True)

            # ---- tail ----
            # stats(n-1) -> rmuh[n-1] = -rho(n)*mu(n-1)
            par_last = (n_steps - 1) & 1          # parity used by last step
            smT = p_sm.tile([128, 4], f32, tag="sm")
            pstT = smT[:, 1:3]
            nc.tensor.matmul(pstT, ones[:], st16[par_last][:],
                             start=True, stop=True)
            mnegT = scr.tile([128, 1], f32, tag="mnegT")
            nc.vector.tensor_scalar(mnegT[:], smT[:, 1:2], -1.0 / H,
                                    None, OP.mult)
            nc.vector.tensor_tensor(rmuh[:, ds(n_steps - 1, 1)],
                                    rho[1 - par_last][:], mnegT[:], OP.mult)
            # final columns n_steps-PRO .. n_steps-1
            for tp in range(n_steps - PRO, n_steps):
                lq8 = scr.tile([128, NJT], f32, tag="lq8T")
                m1 = scr.tile([128, NJT], f32, tag="m1T")
                o1 = scr.tile([128, NJT], f32, tag="o1T")
                nc.vector.tensor_scalar(
                    lq8[:], liqs[:, ds(tp + 1, NJT, SP1)],
                    rh[:, ds(tp, 1)], rmuh[:, ds(tp, 1)], OP.mult, OP.add)
                nc.vector.tensor_tensor(m1[:], lq8[:], lng[:], OP.mult)
                nc.vector.tensor_tensor(m1[:], m1[:], lnb[:], OP.add)
                nc.vector.tensor_scalar(o1[:], m1[:], SCALE, None, OP.mult)
                nc.vector.scalar_tensor_tensor(
                    outb[:, ds(tp, NJT, S)], sc16[:, ds(tp, NJT, S)],
                    1.0 - SCALE, o1[:], OP.mult, OP.add)

            nc.sync.dma_start(outs["outb"], outb[:])
            if "dbg_zr" in outs:
                zdump = st.tile([128, NJT], f32, name="zdump")
                nc.vector.tensor_copy(zdump[:], last_u1[0][:])
                nc.sync.dma_start(outs["dbg_zr"], zdump[:])
            if "dbg_rh" in outs:
                nc.sync.dma_start(outs["dbg_rh"], rh[:])
                nc.sync.dma_start(outs["dbg_rmuh"], rmuh[:])
                nc.sync.dma_start(outs["dbg_liqs"], liqs[:])

    return kernel_fn


def _prep_in_maps(hidden_states, conv_w, W_rec, tau_w1, tau_b1, tau_w2, tau_b2,
                  ln_g, ln_b, w_dt_name="float16"):
    """Host-side staging: per-core input dict (core c gets batch row c%4)."""
    if w_dt_name == "float16":
        np_wdt = np.float16
    else:
        import ml_dtypes
        np_wdt = ml_dtypes.float8_e4m3fn
    x = np.asarray(hidden_states, dtype=np.float32)
    Wfull = np.concatenate([np.asarray(W_rec).T, np.asarray(tau_w1)[H:]], axis=1)
    # [kk, kt, ct, jj]
    wq = Wfull.reshape(NKT, 128, NCT, 128).transpose(1, 0, 2, 3)
    wq = np.ascontiguousarray(wq).astype(np_wdt).reshape(128, NKT * NCT * 128)
    w1xh = np.asarray(tau_w1)[:H]  # [H, HID]
    w1x = w1xh.reshape(NKT, 128, NHT, 128).transpose(1, 0, 2, 3)
    w1x = np.ascontiguousarray(w1x, dtype=np.float16).reshape(128, NKT * NHT * 128)
    w2 = np.ascontiguousarray(
        np.asarray(tau_w2)[:, 0].reshape(NHT, 128).T, dtype=np.float32)
    b1 = np.ascontiguousarray(
        np.asarray(tau_b1).reshape(NHT, 128).T, dtype=np.float32)
    b2h = np.full((128, 1), 0.5 * float(np.asarray(tau_b2)[0]), dtype=np.float32)
    cw = np.ascontiguousarray(
        np.asarray(conv_w).reshape(NJT, 128, K).transpose(1, 0, 2),
        dtype=np.float32).reshape(128, NJT * K)
    lng = np.ascontiguousarray(
        np.asarray(ln_g).reshape(NJT, 128).T, dtype=np.float32)
    lnb = np.ascontiguousarray(
        np.asarray(ln_b).reshape(NJT, 128).T, dtype=np.float32)
    ones = np.ones((128, 128), dtype=np.float16)

    shared = dict(wq=wq, w1x=w1x, w2=w2, b1=b1, b2h=b2h, cw=cw, lng=lng,
                  lnb=lnb, ones=ones)
    in_maps = []
    for c in range(8):
        b = c % B
        xtb = np.ascontiguousarray(
            x[b].T.reshape(NJT, 128, S).transpose(1, 0, 2),
            dtype=np.float16).reshape(128, NJT * S)
        m = dict(shared)
        m["xt"] = xtb
        in_maps.append(m)
    return in_maps


def _in_specs(w_dt_name="float16"):
    from concourse import mybir
    DT = mybir.dt
    wdt = getattr(DT, w_dt_name)
    return {
        "wq": ((128, NKT * NCT * 128), wdt),
        "w1x": ((128, NKT * NHT * 128), DT.float16),
        "xt": ((128, NJT * S), DT.float16),
        "w2": ((128, NHT), DT.float32),
        "b1": ((128, NHT), DT.float32),
        "b2h": ((128, 1), DT.float32),
        "cw": ((128, NJT * K), DT.float32),
        "lng": ((128, NJT), DT.float32),
        "lnb": ((128, NJT), DT.float32),
        "ones": ((128, 128), DT.float16),
    }


def _run_spmd(kernel_fn, in_specs, out_specs, in_maps, num_cores=8, trace=False,
              sim_only=False):
    from concourse import bacc, tile
    from concourse.bass_interp import MultiCoreSim

    nc = bacc.Bacc(
        "TRN2",
        target_bir_lowering=False,
        debug=False,
        enable_asserts=True,
        num_devices=num_cores,
    )
    in_tiles = {
        name: nc.dram_tensor(name, list(shape), dt, kind="ExternalInput").ap()
        for name, (shape, dt) in in_specs.items()
    }
    out_tiles = {
        name: nc.dram_tensor(name, list(shape), dt, kind="ExternalOutput").ap()
        for name, (shape, dt) in out_specs.items()
    }
    with tile.TileContext(nc, trace_sim=True) as tc:
        kernel_fn(tc, out_tiles, in_tiles)
    nc.compile()

    sim = MultiCoreSim(nc, num_cores=num_cores, trace=True)
    for i, core in sim.cores.items():
        for name, arr in in_maps[i].items():
            core.tensor(name)[:] = arr
    if sim_only:
        sim.simulate()
        return sim
    return sim.run_on_hw_raw(trace=trace)


def run_on_device(inputs, n_steps=S, unroll=16, trace=False, w_dt_name="float16",
                  sim_only=False, num_cores=8):
    from concourse import mybir
    DT = mybir.dt
    in_maps = _prep_in_maps(**inputs, w_dt_name=w_dt_name)[:num_cores]
    kernel_fn = _build_kernel(n_steps, unroll=unroll, w_dt_name=w_dt_name)
    out_specs = {"outb": ((128, NJT * S), DT.float16)}
    if sim_only:
        out_specs["dbg_zr"] = ((128, NJT), DT.float32)
        out_specs["dbg_rh"] = ((128, S), DT.float32)
        out_specs["dbg_rmuh"] = ((128, S), DT.float32)
        out_specs["dbg_liqs"] = ((128, NJT * (S + 1)), DT.float16)
    res = _run_spmd(kernel_fn, _in_specs(w_dt_name), out_specs, in_maps,
                    num_cores=num_cores, trace=trace, sim_only=sim_only)
    if sim_only:
        outs = np.empty((min(num_cores, B), S, H), dtype=np.float32)
        for b in range(outs.shape[0]):
            o = np.asarray(res.cores[b].tensor("outb")).astype(np.float32)
            outs[b] = o.reshape(128, NJT, S).transpose(2, 1, 0).reshape(S, H)
        return outs, res
    outs = np.empty((B, S, H), dtype=np.float32)
    for b in range(B):
        o = np.asarray(res.results[b]["outb"]).astype(np.float32)
        outs[b] = o.reshape(128, NJT, S).transpose(2, 1, 0).reshape(S, H)
    return outs, res


def kernel(**inputs):
    out, _ = run_on_device(inputs)
    return out
